# revision 26
# baseline (speedup 1.0000x reference)
"""Trainium2 Bass kernel for nn_CantorModalityFusion.

Sharding: 8 cores = (batch b in 0..3) x (position parity in 0..1).
Each core handles batch b, positions s = par, par+2, ... (1024 positions).
The computation is per-(b, s) independent -> no collectives.

The per-modality input projection is folded into the QKV weights on the
host (Wf = Wp @ W_m, beta = Wp @ (b_m + emb_m) + b_p), so the device
computes q/k/v for each modality directly from the raw modality input
(contraction over dim_m instead of D). Weights and x stream in bf16
(fp32 PSUM accumulation); q/k/v, scores, softmax, fused accumulation
stay fp32/bf16 mixed; output is written bf16.

v3 layout: chunk-outer loops process BOTH 512-position blocks under one
weight load, so each weight byte is fetched once per iteration (~35MB
instead of ~66MB of HBM traffic per core). The repeat loop (timing) uses
staggered_reset so iterations overlap point-to-point instead of through
an all-engine barrier.

Per iteration:
  P1: for c: q.T/k.T chains for both blocks from x; scores accumulate
      into 3 pinned PSUM banks (block0 in partitions 0-63, block1 in
      64-127 via col tile_position)                              [PE+DVE]
  SM: softmax per block, DVE reading scores straight from PSUM  [DVE+ACT]
  A16: per-source summed attn, 4 sources packed into one PSUM bank at
      partition offsets {0,32,64,96} via col tile_position       [PE]
  P2: for c: v.T chains both blocks; Abc = bcast(A16) via row-packed
      16-contraction matmuls; fused.T[c] = sum_r Abc_r * v.T[r] [PE+DVE]
  D:  y.T = Wo.T.T @ fused.T (+ bo) per 128-feature chunk        [PE+ACT]
"""

import os
import sys

import numpy as np

sys.path.insert(0, "/opt/trn_rl_repo")

import ml_dtypes

import concourse.bacc as bacc
import concourse.mybir as mybir
from concourse import tile
from concourse.bass_utils import run_bass_kernel_spmd

F32 = mybir.dt.float32
F32R = mybir.dt.float32r
BF16 = mybir.dt.bfloat16
AF = mybir.ActivationFunctionType
ALU = mybir.AluOpType

B, S, D, H, HD = 4, 2048, 1024, 16, 64
M, WIN = 4, 3
MOD = [("text", 768, 2048), ("image", 1024, 1024), ("audio", 512, 1500), ("video", 2048, 512)]
ROUTES = [[0, 1, 2], [0, 1, 2], [2, 3, 0], [3, 2, 0]]
PAIRS = [(m, w, ROUTES[m][w]) for m in range(M) for w in range(WIN)]
SRC = {r: [(m, w) for (m, w, rr) in PAIRS if rr == r] for r in range(M)}
PAIR_IDX = {(m, w): m * WIN + w for m in range(M) for w in range(WIN)}

NPOS = S // 2
BLK = 512
NBLK = NPOS // BLK
NCH = D // 128                           # 8 output feature chunks
NLOC = [sl // 2 for (_, _, sl) in MOD]   # 1024, 512, 750, 256
NK = [dim // 128 for (_, dim, _) in MOD]  # 6, 8, 4, 16 input chunks
SNK = sum(NK)                             # 34
KOFF = [sum(NK[:m]) for m in range(M)]    # dk offset of modality m

_BUILD_CACHE = {}
_PROBE = os.environ.get("PROBE", "")


def n_active(m, blk):
    return max(0, min(BLK, NLOC[m] - blk * BLK))


def build(scale, repeat=1):
    key = (float(scale), repeat)
    if key in _BUILD_CACHE:
        return _BUILD_CACHE[key]
    nc = bacc.Bacc("TRN2", target_bir_lowering=False, debug=False)

    # x pre-permuted on the host: [p(din%128), dk, pos] -> one DMA per m
    xT = [nc.dram_tensor(f"xT{m}", [128, NK[m], NLOC[m]], BF16,
                         kind="ExternalInput") for m in range(M)]
    # fused q/k weights: [c, p(din%128), ti(q/k), dk, j(dout%128)]
    # all modalities' weights packed along dk: one DMA per (chunk, q/k|v)
    Wqk = nc.dram_tensor("Wqk", [NCH, 128, 2, SNK, 128], BF16,
                         kind="ExternalInput")
    Wvf = nc.dram_tensor("Wvf", [NCH, 128, SNK, 128], BF16,
                         kind="ExternalInput")
    WoT = nc.dram_tensor("WoT", [NCH, 128, NCH, 128], BF16, kind="ExternalInput")
    betaqk_d = nc.dram_tensor("betaqk", [128, M, 2, NCH], F32, kind="ExternalInput")
    betav_d = nc.dram_tensor("betav", [128, M, NCH], F32, kind="ExternalInput")
    bo_d = nc.dram_tensor("bo", [128, NCH], F32, kind="ExternalInput")
    selw_d = nc.dram_tensor("selw", [128, 127], BF16, kind="ExternalInput")
    selA_d = nc.dram_tensor("selA", [64, M * WIN, 16], BF16, kind="ExternalInput")
    selB_d = nc.dram_tensor("selB", [128, NCH, 128], BF16, kind="ExternalInput")
    yT = nc.dram_tensor("yT", [D, NPOS], BF16, kind="ExternalOutput")

    # per-block active modalities / pairs
    nact = [[n_active(m, b) for m in range(M)] for b in range(NBLK)]
    act_m = [[m for m in range(M) if nact[b][m] > 0] for b in range(NBLK)]
    act_pairs = [[(m, w, r) for (m, w, r) in PAIRS
                  if nact[b][m] > 0 and nact[b][r] > 0] for b in range(NBLK)]
    n_sc = {(w, b): sum(1 for (m, w2, r) in act_pairs[b] if w2 == w) * NCH
            for w in range(WIN) for b in range(NBLK)}

    unroll = 4 if repeat % 4 == 0 else (2 if repeat > 1 else 1)
    assert repeat % unroll == 0
    trips = repeat // unroll

    with tile.TileContext(nc) as tc:
        with (
            tc.tile_pool(name="const", bufs=1) as cpool,
            tc.tile_pool(name="wq", bufs=2) as wqpool,
            tc.tile_pool(name="wo", bufs=2) as wopool,
            tc.tile_pool(name="xt", bufs=1) as xtpool,
            tc.tile_pool(name="qk", bufs=1) as qkpool,
            tc.tile_pool(name="pr", bufs=2) as prpool,
            tc.tile_pool(name="sm", bufs=1) as smpool,
            tc.tile_pool(name="fz", bufs=1) as fzpool,
            tc.tile_pool(name="yo", bufs=2) as yopool,
            tc.tile_pool(name="ps", bufs=1, space="PSUM") as pspool,
        ):
            def psum(i):
                return pspool.tile([128, BLK], F32, tag=f"a{i}", name=f"ps_a{i}")

            # ---- constants ----
            selw = cpool.tile([128, 127], BF16, tag="selw")
            nc.sync.dma_start(selw[:], selw_d[:])
            selA = cpool.tile([64, M * WIN, 16], BF16, tag="selA")
            nc.sync.dma_start(selA[:], selA_d[:])
            selB = cpool.tile([128, NCH, 128], BF16, tag="selB")
            nc.sync.dma_start(selB[:], selB_d[:])
            betaqk = cpool.tile([128, M, 2, NCH], F32, tag="betaqk")
            nc.sync.dma_start(betaqk[:], betaqk_d[:])
            betav = cpool.tile([128, M, NCH], F32, tag="betav")
            nc.sync.dma_start(betav[:], betav_d[:])
            bo = cpool.tile([128, NCH], F32, tag="bo")
            nc.sync.dma_start(bo[:], bo_d[:])

            qk_ctr = [0]
            pr_ctr = [0]

            def emit_D(dc, fz_tiles):
                if _PROBE == "qkv":
                    return
                wsl = wopool.tile([128, NCH, 128], BF16,
                                  tag="wo", name="wsld")
                nc.sync.dma_start(wsl[:], WoT[dc])
                for b in range(NBLK):
                    acc = psum(6 + b)
                    for dk in range(NCH):
                        nc.tensor.matmul(
                            acc[:], wsl[:, dk, :], fz_tiles[b][:, dk, :],
                            start=(dk == 0), stop=(dk == NCH - 1),
                            skip_group_check=True)
                    yo = yopool.tile([128, BLK], BF16, tag="yo")
                    nc.vector.tensor_scalar_add(yo[:], acc[:],
                                                bo[:, dc:dc + 1])
                    nc.gpsimd.dma_start(
                        yT[dc * 128:(dc + 1) * 128,
                           b * BLK:(b + 1) * BLK], yo[:])

            def emit_iter(fz_this, fz_prev):
                """One full iteration. fz_prev (if set) is the previous
                iteration's fused output: its projection is interleaved into
                pass 1 so its PE work covers the x/weight reload window."""
                # x rides the gpsimd (SWDGE) queue so its cross-iteration WAR
                # waits never block the weight stream on the sync HWDGE ring.
                xt = {}
                for m in range(M):
                    t = xtpool.tile([128, NK[m], NLOC[m]], BF16,
                                    tag=f"x{m}", name="xtile")
                    nc.gpsimd.dma_start(t[:], xT[m][:])
                    xt[m] = t

                # ---------- pass 1: q, k, scores ----------
                # scores psum: tag a3+w, block0 rows 0-63, block1 rows 64-127
                sc_ps = [psum(3 + w) for w in range(WIN)]
                c_sc = {(w, b): 0 for w in range(WIN) for b in range(NBLK)}

                def emit_qk(c):
                    qk_t = {}
                    wqk = wqpool.tile([128, 2, SNK, 128], BF16,
                                      tag="wqk", name="wqk")
                    for m in range(M):
                        nc.sync.dma_start(
                            wqk[:, :, KOFF[m]:KOFF[m] + NK[m], :],
                            Wqk[c][:, :, KOFF[m]:KOFF[m] + NK[m], :])
                    for m in range(M):
                        for ti, tname in enumerate("qk"):
                            for b in range(NBLK):
                                na = nact[b][m]
                                if na == 0:
                                    continue
                                p0 = b * BLK
                                acc = psum(qk_ctr[0] % 3)
                                qk_ctr[0] += 1
                                for dk in range(NK[m]):
                                    nc.tensor.matmul(
                                        acc[:, :na],
                                        wqk[:, ti, KOFF[m] + dk, :],
                                        xt[m][:, dk, p0:p0 + na],
                                        start=(dk == 0), stop=(dk == NK[m] - 1),
                                        skip_group_check=True)
                                t = qkpool.tile([128, BLK], BF16,
                                                tag=f"{tname}{m}{b}_{c % 2}",
                                                name=f"{tname}{m}{b}")
                                if ti == 0:
                                    nc.vector.tensor_scalar_add(
                                        t[:, :na], acc[:, :na],
                                        betaqk[:, m, ti, c:c + 1])
                                else:
                                    nc.scalar.activation(
                                        t[:, :na], acc[:, :na], AF.Identity,
                                        bias=betaqk[:, m, ti, c:c + 1])
                                if na < BLK:
                                    nc.gpsimd.memset(t[:, na:].bitcast(F32), 0.0)
                                qk_t[(tname, m, b)] = t
                    return qk_t

                def emit_scores(c, qk_t):
                    if _PROBE == "qkv":
                        return
                    for b in range(NBLK):
                        for (m, w, r) in act_pairs[b]:
                            i = c_sc[(w, b)]
                            c_sc[(w, b)] += 1
                            # first matmul of a window must cover the full
                            # width (start=True zeroes the tail); later ones
                            # only where q*k can be nonzero
                            nw = BLK if i == 0 else min(nact[b][m], nact[b][r])
                            prod = prpool.tile([128, BLK], BF16, bufs=1,
                                               tag=f"prod{pr_ctr[0] % 6}",
                                               name="prod")
                            pr_ctr[0] += 1
                            nc.vector.tensor_mul(
                                prod[:, :nw], qk_t[("q", m, b)][:, :nw],
                                qk_t[("k", r, b)][:, :nw])
                            off = 62 - (16 * m + 2 * c)
                            nc.tensor.matmul(
                                sc_ps[w][64 * b:64 * b + 64, :nw],
                                selw[:, off:off + 64],
                                prod[:, :nw],
                                start=(i == 0), stop=(i == n_sc[(w, b)] - 1),
                                skip_group_check=True)

                prev = emit_qk(0)
                for c in range(1, NCH):
                    cur = emit_qk(c)
                    emit_scores(c - 1, prev)
                    if fz_prev is not None:
                        emit_D(c - 1, fz_prev)
                    prev = cur
                emit_scores(NCH - 1, prev)
                if fz_prev is not None:
                    emit_D(NCH - 1, fz_prev)

                # ---------- softmax (per block; DVE reads scores PSUM) ----
                attn = {}
                for b in (() if _PROBE == "qkv" else range(NBLK)):
                    act_w = [w for w in range(WIN) if n_sc[(w, b)] > 0]
                    sl = slice(64 * b, 64 * b + 64)
                    # DVE may read only one PSUM operand per instruction:
                    # copy the first window out, then chain maxes
                    mx = smpool.tile([64, BLK], F32, tag=f"mx{b}")
                    nc.vector.tensor_copy(mx[:], sc_ps[act_w[0]][sl, :])
                    for w in act_w[1:]:
                        nc.vector.tensor_tensor(
                            mx[:], mx[:], sc_ps[w][sl, :], op=ALU.max)
                    if len(act_w) < WIN:
                        # empty windows score 0: it participates in the max
                        nc.vector.tensor_scalar_max(mx[:], mx[:], 0.0)
                    for w in range(WIN):
                        a = smpool.tile([64, BLK], BF16, tag=f"at{w}{b}",
                                        name="attn")
                        if w in act_w:
                            ssub = smpool.tile([64, BLK], F32, bufs=2,
                                               tag="ss", name="ssub")
                            nc.vector.tensor_tensor(
                                ssub[:], sc_ps[w][sl, :], mx[:],
                                op=ALU.subtract)
                            nc.scalar.activation(a[:], ssub[:], AF.Exp,
                                                 scale=scale)
                        else:
                            nc.scalar.activation(a[:], mx[:], AF.Exp,
                                                 scale=-scale)
                        attn[(w, b)] = a
                    den = smpool.tile([64, BLK], F32, tag=f"mx{b}", name="den")
                    nc.vector.tensor_add(den[:], attn[(0, b)][:],
                                         attn[(1, b)][:])
                    nc.vector.tensor_add(den[:], den[:], attn[(2, b)][:])
                    rec = smpool.tile([64, BLK], F32R, tag=f"rec{b}")
                    with nc.allow_low_precision(reason="fp32r attn weights"):
                        nc.vector.reciprocal(rec[:], den[:])
                    for w in range(WIN):
                        nc.vector.tensor_mul(attn[(w, b)][:], attn[(w, b)][:],
                                             rec[:])

                # ---------- A16: per-source summed attn, packed in one bank
                # at partition offsets 32r (col tile_position) ----------
                a16sb = {}
                for b in (() if _PROBE == "qkv" else range(NBLK)):
                    a16 = psum(5)
                    for r in range(M):
                        if nact[b][r] == 0:
                            continue
                        srcs = SRC[r]
                        for i, (m, w) in enumerate(srcs):
                            nc.tensor.matmul(
                                a16[32 * r:32 * r + 16, :],
                                selA[:, PAIR_IDX[(m, w)], :],
                                attn[(w, b)][:],
                                start=(i == 0), stop=(i == len(srcs) - 1),
                                skip_group_check=True,
                                tile_position=(0, 32 * r))
                    t = smpool.tile([128, BLK], BF16, tag=f"a16_{b}",
                                    name="a16sb")
                    nc.scalar.activation(t[:], a16[:], AF.Identity)
                    a16sb[b] = t

                # ---------- pass 2: v, Abc, fused ----------
                def emit_v(c):
                    v_t = {}
                    wv = wqpool.tile([128, SNK, 128], BF16,
                                     tag="wv", name="wv")
                    for m in range(M):
                        nc.sync.dma_start(
                            wv[:, KOFF[m]:KOFF[m] + NK[m], :],
                            Wvf[c][:, KOFF[m]:KOFF[m] + NK[m], :])
                    for m in range(M):
                        for b in range(NBLK):
                            na = nact[b][m]
                            if na == 0:
                                continue
                            p0 = b * BLK
                            acc = v_ctr_rot()
                            for dk in range(NK[m]):
                                nc.tensor.matmul(
                                    acc[:, :na], wv[:, KOFF[m] + dk, :],
                                    xt[m][:, dk, p0:p0 + na],
                                    start=(dk == 0), stop=(dk == NK[m] - 1),
                                    skip_group_check=True)
                            t = qkpool.tile([128, BLK], BF16,
                                            tag=f"q{m}{b}_{c % 2}",
                                            name="vt")
                            nc.vector.tensor_scalar_add(
                                t[:, :na], acc[:, :na],
                                betav[:, m, c:c + 1])
                            if na < BLK:
                                nc.gpsimd.memset(t[:, na:].bitcast(F32), 0.0)
                            v_t[(m, b)] = t
                    return v_t

                VROT = [0, 1, 2, 6, 7]
                def v_ctr_rot():
                    i = VROT[qk_ctr[0] % 5]
                    qk_ctr[0] += 1
                    return psum(i)

                ab_ctr = [0]

                def emit_fused(c, v_t):
                    if _PROBE == "qkv":
                        return
                    for b in range(NBLK):
                        rs = act_m[b]
                        # row-packed broadcast matmuls, two at a time
                        ab_ps = {}
                        for ri in range(0, len(rs), 2):
                            for j, r in enumerate(rs[ri:ri + 2]):
                                ab = psum(3 + (ab_ctr[0] % 3))
                                ab_ctr[0] += 1
                                nc.tensor.matmul(
                                    ab[:], selB[32 * r:32 * r + 16, c, :],
                                    a16sb[b][32 * r:32 * r + 16, :],
                                    start=True, stop=True,
                                    skip_group_check=True,
                                    tile_position=(32 * r, 0))
                                ab_ps[r] = ab
                        r0 = rs[0]
                        accv = prpool.tile([128, BLK], F32, bufs=1,
                                           tag="f0", name="accv")
                        nc.vector.tensor_mul(accv[:], ab_ps[r0][:],
                                             v_t[(r0, b)][:])
                        if len(rs) == 1:
                            nc.vector.tensor_copy(fz_this[b][:, c, :], accv[:])
                        for j, r in enumerate(rs[1:]):
                            tmp = prpool.tile([128, BLK], F32, bufs=1,
                                              tag="f1", name="tmp")
                            nc.vector.tensor_mul(tmp[:], ab_ps[r][:],
                                                 v_t[(r, b)][:])
                            last = (j == len(rs) - 2)
                            nc.vector.tensor_add(
                                fz_this[b][:, c, :] if last else accv[:],
                                accv[:], tmp[:])

                vbuf = {0: emit_v(0), 1: emit_v(1)}
                for c in range(NCH):
                    emit_fused(c, vbuf.pop(c))
                    if c + 2 < NCH:
                        vbuf[c + 2] = emit_v(c + 2)

            import contextlib
            rep_cm = (tc.For_i(0, trips, 1,
                               hint_engines=(mybir.EngineType.PE,
                                             mybir.EngineType.Activation,
                                             mybir.EngineType.DVE,
                                             mybir.EngineType.SP,
                                             mybir.EngineType.Pool),
                               staggered_reset=True)
                      if trips > 1 else contextlib.nullcontext())

            with rep_cm:
                # two fused-output buffer sets, alternating per unroll; the
                # D projection of set s is interleaved into the NEXT
                # iteration's pass 1 (cross-trip for the last unroll)
                fzsets = [[fzpool.tile([128, NCH, BLK], BF16, tag=f"fz{s}{b}",
                                       name=f"fz{s}{b}") for b in range(NBLK)]
                          for s in range(min(unroll, 2))]
                for u in range(unroll):
                    fz_prev = (fzsets[(u + 1) % 2] if repeat > 1 else None)
                    emit_iter(fzsets[u % 2] if repeat > 1 else fzsets[0],
                              fz_prev)

            # ---------- D: final output projection ----------
            # In the repeat loop each iteration's D is handled by the NEXT
            # iteration's pass-1 slots; only the last one remains here.
            # (For repeat=1 this is the only projection.)
            for dc in range(NCH):
                emit_D(dc, fzsets[(unroll - 1) % 2 if repeat > 1 else 0])

    nc.compile()
    _BUILD_CACHE[key] = nc
    return nc


def make_selw():
    sw = np.zeros((128, 127), np.float32)
    for p in range(128):
        sw[p, 62 + p // 64] = 1.0
    return sw


def make_selA():
    sa = np.zeros((64, M * WIN, 16), np.float32)
    for m in range(M):
        for w in range(WIN):
            for h in range(16):
                sa[16 * m + h, m * WIN + w, h] = 1.0
    return sa


def make_selB():
    # [128, NCH, 128]: row 32r+h holds, for chunk c, 0.25 * [h == 2c + j//64]
    sb = np.zeros((128, NCH, 128), np.float32)
    for r in range(M):
        for c in range(NCH):
            for j in range(128):
                sb[32 * r + 2 * c + j // 64, c, j] = 0.25
    return sb


def _vec_tile(v):
    return np.ascontiguousarray(np.asarray(v, np.float32).reshape(NCH, 128).T)


def _wf_tiles(Wf):
    """[D(out), dim(in)] fused weight -> [c, p(din%128), dk, j(dout%128)] bf16."""
    wt = np.asarray(Wf, np.float32).T                  # [din, dout]
    nk = wt.shape[0] // 128
    wt = wt.reshape(nk, 128, NCH, 128)                 # [dk, p, c, j]
    return wt.transpose(2, 1, 0, 3)                    # [c, p, dk, j]


def prepare_in_maps(inputs):
    names = [mm[0] for mm in MOD]
    emb = np.asarray(inputs["mod_emb"], np.float32)
    Wp = {pn: np.asarray(inputs[f"W{pn}"], np.float32) for pn in "qkvo"}
    bp = {pn: np.asarray(inputs[f"b{pn}"], np.float32) for pn in "qkvo"}

    shared = {}
    betaqk = np.zeros((128, M, 2, NCH), np.float32)
    betav = np.zeros((128, M, NCH), np.float32)
    tq, tk, tv = {}, {}, {}
    for i, nm in enumerate(names):
        Wm = np.asarray(inputs[f"W_{nm}"], np.float32)     # [D, dim]
        bm = np.asarray(inputs[f"b_{nm}"], np.float32) + emb[i]
        tq[i] = _wf_tiles(Wp["q"] @ Wm)
        tk[i] = _wf_tiles(Wp["k"] @ Wm)
        tv[i] = _wf_tiles(Wp["v"] @ Wm)
        betaqk[:, i, 0, :] = _vec_tile(Wp["q"] @ bm + bp["q"])
        betaqk[:, i, 1, :] = _vec_tile(Wp["k"] @ bm + bp["k"])
        betav[:, i, :] = _vec_tile(Wp["v"] @ bm + bp["v"])
    shared["betaqk"] = betaqk
    shared["betav"] = betav

    # pack all modalities along dk: [NCH, 128, 2, SNK, 128] / [NCH, 128, SNK, 128]
    wqk_all = np.concatenate(
        [np.stack([tq[m], tk[m]], axis=2) for m in range(M)], axis=3)
    shared["Wqk"] = np.ascontiguousarray(wqk_all).astype(ml_dtypes.bfloat16)
    wv_all = np.concatenate([tv[m] for m in range(M)], axis=2)
    shared["Wvf"] = np.ascontiguousarray(wv_all).astype(ml_dtypes.bfloat16)

    wo = Wp["o"].T.reshape(NCH, 128, NCH, 128)             # [dk, p, c, j]
    shared["WoT"] = np.ascontiguousarray(
        wo.transpose(2, 1, 0, 3)).astype(ml_dtypes.bfloat16)
    shared["bo"] = _vec_tile(bp["o"])
    shared["selw"] = make_selw().astype(ml_dtypes.bfloat16)
    shared["selA"] = make_selA().astype(ml_dtypes.bfloat16)
    shared["selB"] = make_selB().astype(ml_dtypes.bfloat16)

    in_maps = []
    for core in range(8):
        b, par = core // 2, core % 2
        im = dict(shared)
        for i, nm in enumerate(names):
            x = np.asarray(inputs[nm], np.float32)[b, par::2][:NLOC[i]]
            xt = x.T.reshape(NK[i], 128, NLOC[i]).transpose(1, 0, 2)
            im[f"xT{i}"] = np.ascontiguousarray(xt).astype(ml_dtypes.bfloat16)
        in_maps.append(im)
    return in_maps


def kernel(**inputs):
    inputs = {k: np.asarray(v) for k, v in inputs.items()}
    scale = float(1.0 / (np.sqrt(HD) * abs(float(inputs["temperature"]))))
    nc = build(scale, repeat=1)
    in_maps = prepare_in_maps(inputs)
    res = run_bass_kernel_spmd(nc, in_maps, list(range(8)))
    out = np.zeros((B, S, D), np.float32)
    for core in range(8):
        b, par = core // 2, core % 2
        out[b, par::2, :] = np.asarray(res.results[core]["yT"],
                                       np.float32).T
    return out


# revision 28
# speedup vs baseline: 1.2531x; 1.2531x over previous
"""Trainium2 Bass kernel for nn_CantorModalityFusion.

Sharding: 8 cores = (batch b in 0..3) x (position parity in 0..1).
Each core handles batch b, positions s = par, par+2, ... (1024 positions).
The computation is per-(b, s) independent -> no collectives.

The per-modality input projection is folded into the QKV weights on the
host (Wf = Wp @ W_m, beta = Wp @ (b_m + emb_m) + b_p), so the device
computes q/k/v for each modality directly from the raw modality input
(contraction over dim_m instead of D). Weights and x stream in bf16
(fp32 PSUM accumulation); q/k/v, scores, softmax, fused accumulation
stay fp32/bf16 mixed; output is written bf16.

v3 layout: chunk-outer loops process BOTH 512-position blocks under one
weight load, so each weight byte is fetched once per iteration (~35MB
instead of ~66MB of HBM traffic per core). The repeat loop (timing) uses
staggered_reset so iterations overlap point-to-point instead of through
an all-engine barrier.

Per iteration:
  P1: for c: q.T/k.T chains for both blocks from x; scores accumulate
      into 3 pinned PSUM banks (block0 in partitions 0-63, block1 in
      64-127 via col tile_position)                              [PE+DVE]
  SM: softmax per block, DVE reading scores straight from PSUM  [DVE+ACT]
  A16: per-source summed attn, 4 sources packed into one PSUM bank at
      partition offsets {0,32,64,96} via col tile_position       [PE]
  P2: for c: v.T chains both blocks; Abc = bcast(A16) via row-packed
      16-contraction matmuls; fused.T[c] = sum_r Abc_r * v.T[r] [PE+DVE]
  D:  y.T = Wo.T.T @ fused.T (+ bo) per 128-feature chunk        [PE+ACT]
"""

import os
import sys

import numpy as np

sys.path.insert(0, "/opt/trn_rl_repo")

import ml_dtypes

import concourse.bacc as bacc
import concourse.mybir as mybir
from concourse import tile
from concourse.bass_utils import run_bass_kernel_spmd

F32 = mybir.dt.float32
F32R = mybir.dt.float32r
BF16 = mybir.dt.bfloat16
AF = mybir.ActivationFunctionType
ALU = mybir.AluOpType

B, S, D, H, HD = 4, 2048, 1024, 16, 64
M, WIN = 4, 3
MOD = [("text", 768, 2048), ("image", 1024, 1024), ("audio", 512, 1500), ("video", 2048, 512)]
ROUTES = [[0, 1, 2], [0, 1, 2], [2, 3, 0], [3, 2, 0]]
PAIRS = [(m, w, ROUTES[m][w]) for m in range(M) for w in range(WIN)]
SRC = {r: [(m, w) for (m, w, rr) in PAIRS if rr == r] for r in range(M)}
PAIR_IDX = {(m, w): m * WIN + w for m in range(M) for w in range(WIN)}
# A16 source chains grouped by window: one matmul sums all same-window
# contributions to source r (selection matrices merged host-side)
A16_GRPS = {}
_gi = 0
for _r in range(M):
    _by_w = {}
    for (_m, _w) in SRC[_r]:
        _by_w.setdefault(_w, []).append(_m)
    A16_GRPS[_r] = []
    for _w, _ms in sorted(_by_w.items()):
        A16_GRPS[_r].append((_gi, _w))
        _gi += 1
NGRP = _gi

NPOS = S // 2
BLK = 512
NBLK = NPOS // BLK
NCH = D // 128                           # 8 output feature chunks
NLOC = [sl // 2 for (_, _, sl) in MOD]   # 1024, 512, 750, 256
NK = [dim // 128 for (_, dim, _) in MOD]  # 6, 8, 4, 16 input chunks
SNK = sum(NK)                             # 34
KOFF = [sum(NK[:m]) for m in range(M)]    # dk offset of modality m

_BUILD_CACHE = {}
_PROBE = os.environ.get("PROBE", "")


def n_active(m, blk):
    return max(0, min(BLK, NLOC[m] - blk * BLK))


def build(scale, repeat=1):
    key = (float(scale), repeat)
    if key in _BUILD_CACHE:
        return _BUILD_CACHE[key]
    nc = bacc.Bacc("TRN2", target_bir_lowering=False, debug=False)

    # x pre-permuted on the host: [p(din%128), dk, pos] -> one DMA per m
    xT = [nc.dram_tensor(f"xT{m}", [128, NK[m], NLOC[m]], BF16,
                         kind="ExternalInput") for m in range(M)]
    # fused q/k weights: [c, p(din%128), ti(q/k), dk, j(dout%128)]
    # all modalities' weights packed along dk: one DMA per (chunk, q/k|v)
    Wqk = nc.dram_tensor("Wqk", [NCH, 128, 2, SNK, 128], BF16,
                         kind="ExternalInput")
    Wvf = nc.dram_tensor("Wvf", [NCH, 128, SNK, 128], BF16,
                         kind="ExternalInput")
    WoT = nc.dram_tensor("WoT", [NCH, 128, NCH, 128], BF16, kind="ExternalInput")
    betaqk_d = nc.dram_tensor("betaqk", [128, M, 2, NCH], F32, kind="ExternalInput")
    betav_d = nc.dram_tensor("betav", [128, M, NCH], F32, kind="ExternalInput")
    bo_d = nc.dram_tensor("bo", [128, NCH], F32, kind="ExternalInput")
    selw_d = nc.dram_tensor("selw", [128, 127], BF16, kind="ExternalInput")
    selA_d = nc.dram_tensor("selA", [64, NGRP, 16], BF16, kind="ExternalInput")
    selB_d = nc.dram_tensor("selB", [128, NCH, 128], BF16, kind="ExternalInput")
    yT = nc.dram_tensor("yT", [D, NPOS], BF16, kind="ExternalOutput")

    # per-block active modalities / pairs
    nact = [[n_active(m, b) for m in range(M)] for b in range(NBLK)]
    act_m = [[m for m in range(M) if nact[b][m] > 0] for b in range(NBLK)]
    act_pairs = [[(m, w, r) for (m, w, r) in PAIRS
                  if nact[b][m] > 0 and nact[b][r] > 0] for b in range(NBLK)]
    n_sc = {(w, b): sum(1 for (m, w2, r) in act_pairs[b] if w2 == w) * NCH
            for w in range(WIN) for b in range(NBLK)}

    unroll = 4 if repeat % 4 == 0 else (2 if repeat > 1 else 1)
    assert repeat % unroll == 0
    trips = repeat // unroll

    with tile.TileContext(nc) as tc:
        with (
            tc.tile_pool(name="const", bufs=1) as cpool,
            tc.tile_pool(name="wq", bufs=2) as wqpool,
            tc.tile_pool(name="wo", bufs=2) as wopool,
            tc.tile_pool(name="xt", bufs=1) as xtpool,
            tc.tile_pool(name="qk", bufs=1) as qkpool,
            tc.tile_pool(name="pr", bufs=2) as prpool,
            tc.tile_pool(name="sm", bufs=1) as smpool,
            tc.tile_pool(name="fz", bufs=1) as fzpool,
            tc.tile_pool(name="yo", bufs=2) as yopool,
            tc.tile_pool(name="ps", bufs=1, space="PSUM") as pspool,
        ):
            def psum(i):
                return pspool.tile([128, BLK], F32, tag=f"a{i}", name=f"ps_a{i}")

            # ---- constants ----
            selw = cpool.tile([128, 127], BF16, tag="selw")
            nc.sync.dma_start(selw[:], selw_d[:])
            selA = cpool.tile([64, NGRP, 16], BF16, tag="selA")
            nc.sync.dma_start(selA[:], selA_d[:])
            selB = cpool.tile([128, NCH, 128], BF16, tag="selB")
            nc.sync.dma_start(selB[:], selB_d[:])
            betaqk = cpool.tile([128, M, 2, NCH], F32, tag="betaqk")
            nc.sync.dma_start(betaqk[:], betaqk_d[:])
            betav = cpool.tile([128, M, NCH], F32, tag="betav")
            nc.sync.dma_start(betav[:], betav_d[:])
            bo = cpool.tile([128, NCH], F32, tag="bo")
            nc.sync.dma_start(bo[:], bo_d[:])

            qk_ctr = [0]
            pr_ctr = [0]

            def emit_D(dc, fz_tiles):
                if _PROBE == "qkv":
                    return
                wsl = wopool.tile([128, NCH, 128], BF16,
                                  tag="wo", name="wsld")
                nc.sync.dma_start(wsl[:], WoT[dc])
                for b in range(NBLK):
                    acc = psum(6 + b)
                    for dk in range(NCH):
                        nc.tensor.matmul(
                            acc[:], wsl[:, dk, :], fz_tiles[b][:, dk, :],
                            start=(dk == 0), stop=(dk == NCH - 1),
                            skip_group_check=True)
                    yo = yopool.tile([128, BLK], BF16, tag="yo")
                    nc.vector.tensor_scalar_add(yo[:], acc[:],
                                                bo[:, dc:dc + 1])
                    nc.gpsimd.dma_start(
                        yT[dc * 128:(dc + 1) * 128,
                           b * BLK:(b + 1) * BLK], yo[:])

            def emit_iter(fz_this, fz_prev):
                """One full iteration. fz_prev (if set) is the previous
                iteration's fused output: its projection is interleaved into
                pass 1 so its PE work covers the x/weight reload window."""
                # x rides the gpsimd (SWDGE) queue so its cross-iteration WAR
                # waits never block the weight stream on the sync HWDGE ring.
                xt = {}
                for m in range(M):
                    t = xtpool.tile([128, NK[m], NLOC[m]], BF16,
                                    tag=f"x{m}", name="xtile")
                    nc.gpsimd.dma_start(t[:], xT[m][:])
                    xt[m] = t

                # ---------- pass 1: q, k, scores ----------
                # scores psum: tag a3+w, block0 rows 0-63, block1 rows 64-127
                sc_ps = [psum(3 + w) for w in range(WIN)]
                c_sc = {(w, b): 0 for w in range(WIN) for b in range(NBLK)}

                def emit_qk(c):
                    qk_t = {}
                    wqk = wqpool.tile([128, 2, SNK, 128], BF16,
                                      tag="wqk", name="wqk")
                    for ti in range(2):
                        for m in range(M):
                            nc.sync.dma_start(
                                wqk[:, ti, KOFF[m]:KOFF[m] + NK[m], :],
                                Wqk[c][:, ti, KOFF[m]:KOFF[m] + NK[m], :])
                    for m in range(M):
                        for ti, tname in enumerate("qk"):
                            for b in range(NBLK):
                                na = nact[b][m]
                                if na == 0:
                                    continue
                                p0 = b * BLK
                                acc = psum(qk_ctr[0] % 3)
                                qk_ctr[0] += 1
                                for dk in range(NK[m]):
                                    nc.tensor.matmul(
                                        acc[:, :na],
                                        wqk[:, ti, KOFF[m] + dk, :],
                                        xt[m][:, dk, p0:p0 + na],
                                        start=(dk == 0), stop=(dk == NK[m] - 1),
                                        skip_group_check=True)
                                t = qkpool.tile([128, BLK], BF16,
                                                tag=f"{tname}{m}{b}_{c % 2}",
                                                name=f"{tname}{m}{b}")
                                nc.scalar.activation(
                                    t[:, :na], acc[:, :na], AF.Identity,
                                    bias=betaqk[:, m, ti, c:c + 1])
                                if na < BLK:
                                    nc.gpsimd.memset(t[:, na:].bitcast(F32), 0.0)
                                qk_t[(tname, m, b)] = t
                    return qk_t

                def emit_scores(c, qk_t):
                    if _PROBE == "qkv":
                        return
                    for b in range(NBLK):
                        for (m, w, r) in act_pairs[b]:
                            i = c_sc[(w, b)]
                            c_sc[(w, b)] += 1
                            # first matmul of a window must cover the full
                            # width (start=True zeroes the tail); later ones
                            # only where q*k can be nonzero
                            nw = BLK if i == 0 else min(nact[b][m], nact[b][r])
                            prod = prpool.tile([128, BLK], BF16, bufs=1,
                                               tag=f"prod{pr_ctr[0] % 6}",
                                               name="prod")
                            pr_ctr[0] += 1
                            nc.vector.tensor_mul(
                                prod[:, :nw], qk_t[("q", m, b)][:, :nw],
                                qk_t[("k", r, b)][:, :nw])
                            off = 62 - (16 * m + 2 * c)
                            nc.tensor.matmul(
                                sc_ps[w][64 * b:64 * b + 64, :nw],
                                selw[:, off:off + 64],
                                prod[:, :nw],
                                start=(i == 0), stop=(i == n_sc[(w, b)] - 1),
                                skip_group_check=True)

                prev = emit_qk(0)
                for c in range(1, NCH):
                    cur = emit_qk(c)
                    emit_scores(c - 1, prev)
                    if fz_prev is not None:
                        emit_D(c - 1, fz_prev)
                    prev = cur
                emit_scores(NCH - 1, prev)
                if fz_prev is not None:
                    emit_D(NCH - 1, fz_prev)

                # ---------- softmax (per block; DVE reads scores PSUM) ----
                attn = {}
                for b in (() if _PROBE == "qkv" else range(NBLK)):
                    act_w = [w for w in range(WIN) if n_sc[(w, b)] > 0]
                    sl = slice(64 * b, 64 * b + 64)
                    # DVE may read only one PSUM operand per instruction:
                    # copy the first window out, then chain maxes
                    mx = smpool.tile([64, BLK], F32, tag=f"mx{b}")
                    nc.vector.tensor_copy(mx[:], sc_ps[act_w[0]][sl, :])
                    for w in act_w[1:]:
                        nc.vector.tensor_tensor(
                            mx[:], mx[:], sc_ps[w][sl, :], op=ALU.max)
                    if len(act_w) < WIN:
                        # empty windows score 0: it participates in the max
                        nc.vector.tensor_scalar_max(mx[:], mx[:], 0.0)
                    for w in range(WIN):
                        a = smpool.tile([64, BLK], BF16, tag=f"at{w}{b}",
                                        name="attn")
                        if w in act_w:
                            ssub = smpool.tile([64, BLK], F32, bufs=2,
                                               tag="ss", name="ssub")
                            nc.vector.tensor_tensor(
                                ssub[:], sc_ps[w][sl, :], mx[:],
                                op=ALU.subtract)
                            nc.scalar.activation(a[:], ssub[:], AF.Exp,
                                                 scale=scale)
                        else:
                            nc.scalar.activation(a[:], mx[:], AF.Exp,
                                                 scale=-scale)
                        attn[(w, b)] = a
                    den = smpool.tile([64, BLK], F32, tag=f"mx{b}", name="den")
                    nc.vector.tensor_add(den[:], attn[(0, b)][:],
                                         attn[(1, b)][:])
                    nc.vector.tensor_add(den[:], den[:], attn[(2, b)][:])
                    rec = smpool.tile([64, BLK], F32R, tag=f"rec{b}")
                    with nc.allow_low_precision(reason="fp32r attn weights"):
                        nc.vector.reciprocal(rec[:], den[:])
                    for w in range(WIN):
                        nc.vector.tensor_mul(attn[(w, b)][:], attn[(w, b)][:],
                                             rec[:])

                # ---------- A16: per-source summed attn, packed in one bank
                # at partition offsets 32r (col tile_position) ----------
                a16sb = {}
                for b in (() if _PROBE == "qkv" else range(NBLK)):
                    a16 = psum(5)
                    for r in range(M):
                        if nact[b][r] == 0:
                            continue
                        grps = A16_GRPS[r]
                        for i, (gi, w) in enumerate(grps):
                            nc.tensor.matmul(
                                a16[32 * r:32 * r + 16, :],
                                selA[:, gi, :],
                                attn[(w, b)][:],
                                start=(i == 0), stop=(i == len(grps) - 1),
                                skip_group_check=True,
                                tile_position=(0, 32 * r))
                    t = smpool.tile([128, BLK], BF16, tag=f"a16_{b}",
                                    name="a16sb")
                    nc.scalar.activation(t[:], a16[:], AF.Identity)
                    a16sb[b] = t

                # ---------- pass 2: v, Abc, fused ----------
                def emit_v(c):
                    v_t = {}
                    wv = wqpool.tile([128, SNK, 128], BF16,
                                     tag="wv", name="wv")
                    for m in range(M):
                        nc.sync.dma_start(
                            wv[:, KOFF[m]:KOFF[m] + NK[m], :],
                            Wvf[c][:, KOFF[m]:KOFF[m] + NK[m], :])
                    for m in range(M):
                        for b in range(NBLK):
                            na = nact[b][m]
                            if na == 0:
                                continue
                            p0 = b * BLK
                            acc = v_ctr_rot()
                            for dk in range(NK[m]):
                                nc.tensor.matmul(
                                    acc[:, :na], wv[:, KOFF[m] + dk, :],
                                    xt[m][:, dk, p0:p0 + na],
                                    start=(dk == 0), stop=(dk == NK[m] - 1),
                                    skip_group_check=True)
                            t = qkpool.tile([128, BLK], BF16,
                                            tag=f"q{m}{b}_{c % 2}",
                                            name="vt")
                            nc.vector.tensor_scalar_add(
                                t[:, :na], acc[:, :na],
                                betav[:, m, c:c + 1])
                            if na < BLK:
                                nc.gpsimd.memset(t[:, na:].bitcast(F32), 0.0)
                            v_t[(m, b)] = t
                    return v_t

                VROT = [0, 1, 2, 6, 7]
                def v_ctr_rot():
                    i = VROT[qk_ctr[0] % 5]
                    qk_ctr[0] += 1
                    return psum(i)

                ab_ctr = [0]

                def emit_fused(c, v_t):
                    if _PROBE == "qkv":
                        return
                    for b in range(NBLK):
                        rs = act_m[b]
                        # row-packed broadcast matmuls, two at a time
                        ab_ps = {}
                        for ri in range(0, len(rs), 2):
                            for j, r in enumerate(rs[ri:ri + 2]):
                                ab = psum(3 + (ab_ctr[0] % 3))
                                ab_ctr[0] += 1
                                nc.tensor.matmul(
                                    ab[:], selB[32 * r:32 * r + 16, c, :],
                                    a16sb[b][32 * r:32 * r + 16, :],
                                    start=True, stop=True,
                                    skip_group_check=True,
                                    tile_position=(32 * r, 0))
                                ab_ps[r] = ab
                        r0 = rs[0]
                        accv = prpool.tile([128, BLK], F32, bufs=1,
                                           tag="f0", name="accv")
                        nc.vector.tensor_mul(accv[:], ab_ps[r0][:],
                                             v_t[(r0, b)][:])
                        if len(rs) == 1:
                            nc.vector.tensor_copy(fz_this[b][:, c, :], accv[:])
                        for j, r in enumerate(rs[1:]):
                            tmp = prpool.tile([128, BLK], F32, bufs=1,
                                              tag="f1", name="tmp")
                            nc.vector.tensor_mul(tmp[:], ab_ps[r][:],
                                                 v_t[(r, b)][:])
                            last = (j == len(rs) - 2)
                            nc.vector.tensor_add(
                                fz_this[b][:, c, :] if last else accv[:],
                                accv[:], tmp[:])

                vbuf = {0: emit_v(0), 1: emit_v(1)}
                for c in range(NCH):
                    emit_fused(c, vbuf.pop(c))
                    if c + 2 < NCH:
                        vbuf[c + 2] = emit_v(c + 2)

            import contextlib
            rep_cm = (tc.For_i(0, trips, 1,
                               hint_engines=(mybir.EngineType.PE,
                                             mybir.EngineType.Activation,
                                             mybir.EngineType.DVE,
                                             mybir.EngineType.SP,
                                             mybir.EngineType.Pool),
                               staggered_reset=True)
                      if trips > 1 else contextlib.nullcontext())

            with rep_cm:
                # two fused-output buffer sets, alternating per unroll; the
                # D projection of set s is interleaved into the NEXT
                # iteration's pass 1 (cross-trip for the last unroll)
                fzsets = [[fzpool.tile([128, NCH, BLK], BF16, tag=f"fz{s}{b}",
                                       name=f"fz{s}{b}") for b in range(NBLK)]
                          for s in range(min(unroll, 2))]
                for u in range(unroll):
                    fz_prev = (fzsets[(u + 1) % 2] if repeat > 1 else None)
                    emit_iter(fzsets[u % 2] if repeat > 1 else fzsets[0],
                              fz_prev)

            # ---------- D: final output projection ----------
            # In the repeat loop each iteration's D is handled by the NEXT
            # iteration's pass-1 slots; only the last one remains here.
            # (For repeat=1 this is the only projection.)
            for dc in range(NCH):
                emit_D(dc, fzsets[(unroll - 1) % 2 if repeat > 1 else 0])

    nc.compile()
    _BUILD_CACHE[key] = nc
    return nc


def make_selw():
    sw = np.zeros((128, 127), np.float32)
    for p in range(128):
        sw[p, 62 + p // 64] = 1.0
    return sw


def make_selA():
    sa = np.zeros((64, NGRP, 16), np.float32)
    for r in range(M):
        by_w = {}
        for (m, w) in SRC[r]:
            by_w.setdefault(w, []).append(m)
        for (gi, w), (w2, ms) in zip(A16_GRPS[r], sorted(by_w.items())):
            assert w == w2
            for m in ms:
                for h in range(16):
                    sa[16 * m + h, gi, h] = 1.0
    return sa


def make_selB():
    # [128, NCH, 128]: row 32r+h holds, for chunk c, 0.25 * [h == 2c + j//64]
    sb = np.zeros((128, NCH, 128), np.float32)
    for r in range(M):
        for c in range(NCH):
            for j in range(128):
                sb[32 * r + 2 * c + j // 64, c, j] = 0.25
    return sb


def _vec_tile(v):
    return np.ascontiguousarray(np.asarray(v, np.float32).reshape(NCH, 128).T)


def _wf_tiles(Wf):
    """[D(out), dim(in)] fused weight -> [c, p(din%128), dk, j(dout%128)] bf16."""
    wt = np.asarray(Wf, np.float32).T                  # [din, dout]
    nk = wt.shape[0] // 128
    wt = wt.reshape(nk, 128, NCH, 128)                 # [dk, p, c, j]
    return wt.transpose(2, 1, 0, 3)                    # [c, p, dk, j]


def prepare_in_maps(inputs):
    names = [mm[0] for mm in MOD]
    emb = np.asarray(inputs["mod_emb"], np.float32)
    Wp = {pn: np.asarray(inputs[f"W{pn}"], np.float32) for pn in "qkvo"}
    bp = {pn: np.asarray(inputs[f"b{pn}"], np.float32) for pn in "qkvo"}

    shared = {}
    betaqk = np.zeros((128, M, 2, NCH), np.float32)
    betav = np.zeros((128, M, NCH), np.float32)
    tq, tk, tv = {}, {}, {}
    for i, nm in enumerate(names):
        Wm = np.asarray(inputs[f"W_{nm}"], np.float32)     # [D, dim]
        bm = np.asarray(inputs[f"b_{nm}"], np.float32) + emb[i]
        tq[i] = _wf_tiles(Wp["q"] @ Wm)
        tk[i] = _wf_tiles(Wp["k"] @ Wm)
        tv[i] = _wf_tiles(Wp["v"] @ Wm)
        betaqk[:, i, 0, :] = _vec_tile(Wp["q"] @ bm + bp["q"])
        betaqk[:, i, 1, :] = _vec_tile(Wp["k"] @ bm + bp["k"])
        betav[:, i, :] = _vec_tile(Wp["v"] @ bm + bp["v"])
    shared["betaqk"] = betaqk
    shared["betav"] = betav

    # pack all modalities along dk: [NCH, 128, 2, SNK, 128] / [NCH, 128, SNK, 128]
    wqk_all = np.concatenate(
        [np.stack([tq[m], tk[m]], axis=2) for m in range(M)], axis=3)
    shared["Wqk"] = np.ascontiguousarray(wqk_all).astype(ml_dtypes.bfloat16)
    wv_all = np.concatenate([tv[m] for m in range(M)], axis=2)
    shared["Wvf"] = np.ascontiguousarray(wv_all).astype(ml_dtypes.bfloat16)

    wo = Wp["o"].T.reshape(NCH, 128, NCH, 128)             # [dk, p, c, j]
    shared["WoT"] = np.ascontiguousarray(
        wo.transpose(2, 1, 0, 3)).astype(ml_dtypes.bfloat16)
    shared["bo"] = _vec_tile(bp["o"])
    shared["selw"] = make_selw().astype(ml_dtypes.bfloat16)
    shared["selA"] = make_selA().astype(ml_dtypes.bfloat16)
    shared["selB"] = make_selB().astype(ml_dtypes.bfloat16)

    in_maps = []
    for core in range(8):
        b, par = core // 2, core % 2
        im = dict(shared)
        for i, nm in enumerate(names):
            x = np.asarray(inputs[nm], np.float32)[b, par::2][:NLOC[i]]
            xt = x.T.reshape(NK[i], 128, NLOC[i]).transpose(1, 0, 2)
            im[f"xT{i}"] = np.ascontiguousarray(xt).astype(ml_dtypes.bfloat16)
        in_maps.append(im)
    return in_maps


def kernel(**inputs):
    inputs = {k: np.asarray(v) for k, v in inputs.items()}
    scale = float(1.0 / (np.sqrt(HD) * abs(float(inputs["temperature"]))))
    nc = build(scale, repeat=1)
    in_maps = prepare_in_maps(inputs)
    res = run_bass_kernel_spmd(nc, in_maps, list(range(8)))
    out = np.zeros((B, S, D), np.float32)
    for core in range(8):
        b, par = core // 2, core % 2
        out[b, par::2, :] = np.asarray(res.results[core]["yT"],
                                       np.float32).T
    return out


# revision 29
# speedup vs baseline: 1.2586x; 1.0044x over previous
"""Trainium2 Bass kernel for nn_CantorModalityFusion.

Sharding: 8 cores = (batch b in 0..3) x (position parity in 0..1).
Each core handles batch b, positions s = par, par+2, ... (1024 positions).
The computation is per-(b, s) independent -> no collectives.

The per-modality input projection is folded into the QKV weights on the
host (Wf = Wp @ W_m, beta = Wp @ (b_m + emb_m) + b_p), so the device
computes q/k/v for each modality directly from the raw modality input
(contraction over dim_m instead of D). Weights and x stream in bf16
(fp32 PSUM accumulation); q/k/v, scores, softmax, fused accumulation
stay fp32/bf16 mixed; output is written bf16.

v3 layout: chunk-outer loops process BOTH 512-position blocks under one
weight load, so each weight byte is fetched once per iteration (~35MB
instead of ~66MB of HBM traffic per core). The repeat loop (timing) uses
staggered_reset so iterations overlap point-to-point instead of through
an all-engine barrier.

Per iteration:
  P1: for c: q.T/k.T chains for both blocks from x; scores accumulate
      into 3 pinned PSUM banks (block0 in partitions 0-63, block1 in
      64-127 via col tile_position)                              [PE+DVE]
  SM: softmax per block, DVE reading scores straight from PSUM  [DVE+ACT]
  A16: per-source summed attn, 4 sources packed into one PSUM bank at
      partition offsets {0,32,64,96} via col tile_position       [PE]
  P2: for c: v.T chains both blocks; Abc = bcast(A16) via row-packed
      16-contraction matmuls; fused.T[c] = sum_r Abc_r * v.T[r] [PE+DVE]
  D:  y.T = Wo.T.T @ fused.T (+ bo) per 128-feature chunk        [PE+ACT]
"""

import os
import sys

import numpy as np

sys.path.insert(0, "/opt/trn_rl_repo")

import ml_dtypes

import concourse.bacc as bacc
import concourse.mybir as mybir
from concourse import tile
from concourse.bass_utils import run_bass_kernel_spmd

F32 = mybir.dt.float32
F32R = mybir.dt.float32r
BF16 = mybir.dt.bfloat16
AF = mybir.ActivationFunctionType
ALU = mybir.AluOpType

B, S, D, H, HD = 4, 2048, 1024, 16, 64
M, WIN = 4, 3
MOD = [("text", 768, 2048), ("image", 1024, 1024), ("audio", 512, 1500), ("video", 2048, 512)]
ROUTES = [[0, 1, 2], [0, 1, 2], [2, 3, 0], [3, 2, 0]]
PAIRS = [(m, w, ROUTES[m][w]) for m in range(M) for w in range(WIN)]
SRC = {r: [(m, w) for (m, w, rr) in PAIRS if rr == r] for r in range(M)}
PAIR_IDX = {(m, w): m * WIN + w for m in range(M) for w in range(WIN)}
# A16 source chains grouped by window: one matmul sums all same-window
# contributions to source r (selection matrices merged host-side)
A16_GRPS = {}
_gi = 0
for _r in range(M):
    _by_w = {}
    for (_m, _w) in SRC[_r]:
        _by_w.setdefault(_w, []).append(_m)
    A16_GRPS[_r] = []
    for _w, _ms in sorted(_by_w.items()):
        A16_GRPS[_r].append((_gi, _w))
        _gi += 1
NGRP = _gi

NPOS = S // 2
BLK = 512
NBLK = NPOS // BLK
NCH = D // 128                           # 8 output feature chunks
NLOC = [sl // 2 for (_, _, sl) in MOD]   # 1024, 512, 750, 256
NK = [dim // 128 for (_, dim, _) in MOD]  # 6, 8, 4, 16 input chunks
SNK = sum(NK)                             # 34
KOFF = [sum(NK[:m]) for m in range(M)]    # dk offset of modality m

_BUILD_CACHE = {}
_PROBE = os.environ.get("PROBE", "")


def n_active(m, blk):
    return max(0, min(BLK, NLOC[m] - blk * BLK))


def build(scale, repeat=1):
    key = (float(scale), repeat)
    if key in _BUILD_CACHE:
        return _BUILD_CACHE[key]
    nc = bacc.Bacc("TRN2", target_bir_lowering=False, debug=False)

    # x pre-permuted on the host: [p(din%128), dk, pos] -> one DMA per m
    xT = [nc.dram_tensor(f"xT{m}", [128, NK[m], NLOC[m]], BF16,
                         kind="ExternalInput") for m in range(M)]
    # fused q/k weights: [c, p(din%128), ti(q/k), dk, j(dout%128)]
    # all modalities' weights packed along dk: one DMA per (chunk, q/k|v)
    Wqk = nc.dram_tensor("Wqk", [NCH, 128, 2, SNK, 128], BF16,
                         kind="ExternalInput")
    Wvf = nc.dram_tensor("Wvf", [NCH, 128, SNK, 128], BF16,
                         kind="ExternalInput")
    WoT = nc.dram_tensor("WoT", [NCH, 128, NCH, 128], BF16, kind="ExternalInput")
    betaqk_d = nc.dram_tensor("betaqk", [128, M, 2, NCH], F32, kind="ExternalInput")
    betav_d = nc.dram_tensor("betav", [128, M, NCH], F32, kind="ExternalInput")
    bo_d = nc.dram_tensor("bo", [128, NCH], F32, kind="ExternalInput")
    selw_d = nc.dram_tensor("selw", [128, 127], BF16, kind="ExternalInput")
    selA_d = nc.dram_tensor("selA", [64, NGRP, 16], BF16, kind="ExternalInput")
    selB_d = nc.dram_tensor("selB", [128, NCH, 128], BF16, kind="ExternalInput")
    yT = nc.dram_tensor("yT", [D, NPOS], BF16, kind="ExternalOutput")

    # per-block active modalities / pairs
    nact = [[n_active(m, b) for m in range(M)] for b in range(NBLK)]
    act_m = [[m for m in range(M) if nact[b][m] > 0] for b in range(NBLK)]
    act_pairs = [[(m, w, r) for (m, w, r) in PAIRS
                  if nact[b][m] > 0 and nact[b][r] > 0] for b in range(NBLK)]
    n_sc = {(w, b): sum(1 for (m, w2, r) in act_pairs[b] if w2 == w) * NCH
            for w in range(WIN) for b in range(NBLK)}

    unroll = 4 if repeat % 4 == 0 else (2 if repeat > 1 else 1)
    assert repeat % unroll == 0
    trips = repeat // unroll

    with tile.TileContext(nc) as tc:
        with (
            tc.tile_pool(name="const", bufs=1) as cpool,
            tc.tile_pool(name="wq", bufs=2) as wqpool,
            tc.tile_pool(name="wo", bufs=2) as wopool,
            tc.tile_pool(name="xt", bufs=1) as xtpool,
            tc.tile_pool(name="qk", bufs=1) as qkpool,
            tc.tile_pool(name="pr", bufs=2) as prpool,
            tc.tile_pool(name="sm", bufs=1) as smpool,
            tc.tile_pool(name="fz", bufs=1) as fzpool,
            tc.tile_pool(name="yo", bufs=2) as yopool,
            tc.tile_pool(name="ps", bufs=1, space="PSUM") as pspool,
        ):
            def psum(i):
                return pspool.tile([128, BLK], F32, tag=f"a{i}", name=f"ps_a{i}")

            # ---- constants ----
            selw = cpool.tile([128, 127], BF16, tag="selw")
            nc.sync.dma_start(selw[:], selw_d[:])
            selA = cpool.tile([64, NGRP, 16], BF16, tag="selA")
            nc.sync.dma_start(selA[:], selA_d[:])
            selB = cpool.tile([128, NCH, 128], BF16, tag="selB")
            nc.sync.dma_start(selB[:], selB_d[:])
            betaqk = cpool.tile([128, M, 2, NCH], F32, tag="betaqk")
            nc.sync.dma_start(betaqk[:], betaqk_d[:])
            betav = cpool.tile([128, M, NCH], F32, tag="betav")
            nc.sync.dma_start(betav[:], betav_d[:])
            bo = cpool.tile([128, NCH], F32, tag="bo")
            nc.sync.dma_start(bo[:], bo_d[:])

            qk_ctr = [0]
            pr_ctr = [0]

            def emit_D(dc, fz_tiles):
                if _PROBE == "qkv":
                    return
                wsl = wopool.tile([128, NCH, 128], BF16,
                                  tag="wo", name="wsld")
                nc.sync.dma_start(wsl[:], WoT[dc])
                for b in range(NBLK):
                    acc = psum(6 + b)
                    for dk in range(NCH):
                        nc.tensor.matmul(
                            acc[:], wsl[:, dk, :], fz_tiles[b][:, dk, :],
                            start=(dk == 0), stop=(dk == NCH - 1),
                            skip_group_check=True)
                    yo = yopool.tile([128, BLK], BF16, tag="yo")
                    nc.vector.tensor_scalar_add(yo[:], acc[:],
                                                bo[:, dc:dc + 1])
                    nc.gpsimd.dma_start(
                        yT[dc * 128:(dc + 1) * 128,
                           b * BLK:(b + 1) * BLK], yo[:])

            def emit_iter(fz_this, fz_prev):
                """One full iteration. fz_prev (if set) is the previous
                iteration's fused output: its projection is interleaved into
                pass 1 so its PE work covers the x/weight reload window."""
                # x rides the gpsimd (SWDGE) queue so its cross-iteration WAR
                # waits never block the weight stream on the sync HWDGE ring.
                xt = {}
                for m in range(M):
                    t = xtpool.tile([128, NK[m], NLOC[m]], BF16,
                                    tag=f"x{m}", name="xtile")
                    nc.gpsimd.dma_start(t[:], xT[m][:])
                    xt[m] = t

                # ---------- pass 1: q, k, scores ----------
                # scores psum: tag a3+w, block0 rows 0-63, block1 rows 64-127
                sc_ps = [psum(3 + w) for w in range(WIN)]
                c_sc = {(w, b): 0 for w in range(WIN) for b in range(NBLK)}

                def emit_qk(c):
                    qk_t = {}
                    wqk = wqpool.tile([128, 2, SNK, 128], BF16,
                                      tag="wqk", name="wqk")
                    for m in range(M):
                        nc.sync.dma_start(
                            wqk[:, :, KOFF[m]:KOFF[m] + NK[m], :],
                            Wqk[c][:, :, KOFF[m]:KOFF[m] + NK[m], :])
                    for m in range(M):
                        for ti, tname in enumerate("qk"):
                            for b in range(NBLK):
                                na = nact[b][m]
                                if na == 0:
                                    continue
                                p0 = b * BLK
                                acc = psum(qk_ctr[0] % 3)
                                qk_ctr[0] += 1
                                for dk in range(NK[m]):
                                    nc.tensor.matmul(
                                        acc[:, :na],
                                        wqk[:, ti, KOFF[m] + dk, :],
                                        xt[m][:, dk, p0:p0 + na],
                                        start=(dk == 0), stop=(dk == NK[m] - 1),
                                        skip_group_check=True)
                                t = qkpool.tile([128, BLK], BF16,
                                                tag=f"{tname}{m}{b}_{c % 2}",
                                                name=f"{tname}{m}{b}")
                                nc.scalar.activation(
                                    t[:, :na], acc[:, :na], AF.Identity,
                                    bias=betaqk[:, m, ti, c:c + 1])
                                if na < BLK:
                                    nc.gpsimd.memset(t[:, na:].bitcast(F32), 0.0)
                                qk_t[(tname, m, b)] = t
                    return qk_t

                def emit_scores(c, qk_t):
                    if _PROBE == "qkv":
                        return
                    for b in range(NBLK):
                        for (m, w, r) in act_pairs[b]:
                            i = c_sc[(w, b)]
                            c_sc[(w, b)] += 1
                            # first matmul of a window must cover the full
                            # width (start=True zeroes the tail); later ones
                            # only where q*k can be nonzero
                            nw = BLK if i == 0 else min(nact[b][m], nact[b][r])
                            prod = prpool.tile([128, BLK], BF16, bufs=1,
                                               tag=f"prod{pr_ctr[0] % 6}",
                                               name="prod")
                            pr_ctr[0] += 1
                            nc.vector.tensor_mul(
                                prod[:, :nw], qk_t[("q", m, b)][:, :nw],
                                qk_t[("k", r, b)][:, :nw])
                            off = 62 - (16 * m + 2 * c)
                            nc.tensor.matmul(
                                sc_ps[w][64 * b:64 * b + 64, :nw],
                                selw[:, off:off + 64],
                                prod[:, :nw],
                                start=(i == 0), stop=(i == n_sc[(w, b)] - 1),
                                skip_group_check=True)

                prev = emit_qk(0)
                for c in range(1, NCH):
                    cur = emit_qk(c)
                    emit_scores(c - 1, prev)
                    if fz_prev is not None:
                        emit_D(c - 1, fz_prev)
                    prev = cur
                emit_scores(NCH - 1, prev)
                if fz_prev is not None:
                    emit_D(NCH - 1, fz_prev)

                # ---------- softmax (per block; DVE reads scores PSUM) ----
                attn = {}
                for b in (() if _PROBE == "qkv" else range(NBLK)):
                    act_w = [w for w in range(WIN) if n_sc[(w, b)] > 0]
                    sl = slice(64 * b, 64 * b + 64)
                    # DVE may read only one PSUM operand per instruction:
                    # copy the first window out, then chain maxes
                    mx = smpool.tile([64, BLK], F32, tag=f"mx{b}")
                    nc.vector.tensor_copy(mx[:], sc_ps[act_w[0]][sl, :])
                    for w in act_w[1:]:
                        nc.vector.tensor_tensor(
                            mx[:], mx[:], sc_ps[w][sl, :], op=ALU.max)
                    if len(act_w) < WIN:
                        # empty windows score 0: it participates in the max
                        nc.vector.tensor_scalar_max(mx[:], mx[:], 0.0)
                    for w in range(WIN):
                        a = smpool.tile([64, BLK], BF16, tag=f"at{w}{b}",
                                        name="attn")
                        if w in act_w:
                            ssub = smpool.tile([64, BLK], F32, bufs=2,
                                               tag="ss", name="ssub")
                            nc.vector.tensor_tensor(
                                ssub[:], sc_ps[w][sl, :], mx[:],
                                op=ALU.subtract)
                            nc.scalar.activation(a[:], ssub[:], AF.Exp,
                                                 scale=scale)
                        else:
                            nc.scalar.activation(a[:], mx[:], AF.Exp,
                                                 scale=-scale)
                        attn[(w, b)] = a
                    den = smpool.tile([64, BLK], F32, tag=f"mx{b}", name="den")
                    nc.vector.tensor_add(den[:], attn[(0, b)][:],
                                         attn[(1, b)][:])
                    nc.vector.tensor_add(den[:], den[:], attn[(2, b)][:])
                    rec = smpool.tile([64, BLK], F32R, tag=f"rec{b}")
                    with nc.allow_low_precision(reason="fp32r attn weights"):
                        nc.vector.reciprocal(rec[:], den[:])
                    for w in range(WIN):
                        nc.vector.tensor_mul(attn[(w, b)][:], attn[(w, b)][:],
                                             rec[:])

                # ---------- A16: per-source summed attn, packed in one bank
                # at partition offsets 32r (col tile_position) ----------
                a16sb = {}
                for b in (() if _PROBE == "qkv" else range(NBLK)):
                    a16 = psum(5)
                    for r in range(M):
                        if nact[b][r] == 0:
                            continue
                        grps = A16_GRPS[r]
                        for i, (gi, w) in enumerate(grps):
                            nc.tensor.matmul(
                                a16[32 * r:32 * r + 16, :],
                                selA[:, gi, :],
                                attn[(w, b)][:],
                                start=(i == 0), stop=(i == len(grps) - 1),
                                skip_group_check=True,
                                tile_position=(0, 32 * r))
                    t = smpool.tile([128, BLK], BF16, tag=f"a16_{b}",
                                    name="a16sb")
                    nc.scalar.activation(t[:], a16[:], AF.Identity)
                    a16sb[b] = t

                # ---------- pass 2: v, Abc, fused ----------
                def emit_v(c):
                    v_t = {}
                    wv = wqpool.tile([128, SNK, 128], BF16,
                                     tag="wv", name="wv")
                    for m in range(M):
                        nc.sync.dma_start(
                            wv[:, KOFF[m]:KOFF[m] + NK[m], :],
                            Wvf[c][:, KOFF[m]:KOFF[m] + NK[m], :])
                    for m in range(M):
                        for b in range(NBLK):
                            na = nact[b][m]
                            if na == 0:
                                continue
                            p0 = b * BLK
                            acc = v_ctr_rot()
                            for dk in range(NK[m]):
                                nc.tensor.matmul(
                                    acc[:, :na], wv[:, KOFF[m] + dk, :],
                                    xt[m][:, dk, p0:p0 + na],
                                    start=(dk == 0), stop=(dk == NK[m] - 1),
                                    skip_group_check=True)
                            t = qkpool.tile([128, BLK], BF16,
                                            tag=f"q{m}{b}_{c % 2}",
                                            name="vt")
                            nc.vector.tensor_scalar_add(
                                t[:, :na], acc[:, :na],
                                betav[:, m, c:c + 1])
                            if na < BLK:
                                nc.gpsimd.memset(t[:, na:].bitcast(F32), 0.0)
                            v_t[(m, b)] = t
                    return v_t

                VROT = [0, 1, 2, 6, 7]
                def v_ctr_rot():
                    i = VROT[qk_ctr[0] % 5]
                    qk_ctr[0] += 1
                    return psum(i)

                ab_ctr = [0]

                def emit_fused(c, v_t):
                    if _PROBE == "qkv":
                        return
                    for b in range(NBLK):
                        rs = act_m[b]
                        # row-packed broadcast matmuls, two at a time
                        ab_ps = {}
                        for ri in range(0, len(rs), 2):
                            for j, r in enumerate(rs[ri:ri + 2]):
                                ab = psum(3 + (ab_ctr[0] % 3))
                                ab_ctr[0] += 1
                                nc.tensor.matmul(
                                    ab[:], selB[32 * r:32 * r + 16, c, :],
                                    a16sb[b][32 * r:32 * r + 16, :],
                                    start=True, stop=True,
                                    skip_group_check=True,
                                    tile_position=(32 * r, 0))
                                ab_ps[r] = ab
                        r0 = rs[0]
                        accv = prpool.tile([128, BLK], F32, bufs=1,
                                           tag="f0", name="accv")
                        nc.vector.tensor_mul(accv[:], ab_ps[r0][:],
                                             v_t[(r0, b)][:])
                        if len(rs) == 1:
                            nc.vector.tensor_copy(fz_this[b][:, c, :], accv[:])
                        for j, r in enumerate(rs[1:]):
                            tmp = prpool.tile([128, BLK], F32, bufs=1,
                                              tag="f1", name="tmp")
                            nc.vector.tensor_mul(tmp[:], ab_ps[r][:],
                                                 v_t[(r, b)][:])
                            last = (j == len(rs) - 2)
                            nc.vector.tensor_add(
                                fz_this[b][:, c, :] if last else accv[:],
                                accv[:], tmp[:])

                vbuf = {0: emit_v(0), 1: emit_v(1)}
                for c in range(NCH):
                    emit_fused(c, vbuf.pop(c))
                    if c + 2 < NCH:
                        vbuf[c + 2] = emit_v(c + 2)

            import contextlib
            rep_cm = (tc.For_i(0, trips, 1,
                               hint_engines=(mybir.EngineType.PE,
                                             mybir.EngineType.Activation,
                                             mybir.EngineType.DVE,
                                             mybir.EngineType.SP,
                                             mybir.EngineType.Pool),
                               staggered_reset=True)
                      if trips > 1 else contextlib.nullcontext())

            with rep_cm:
                # two fused-output buffer sets, alternating per unroll; the
                # D projection of set s is interleaved into the NEXT
                # iteration's pass 1 (cross-trip for the last unroll)
                fzsets = [[fzpool.tile([128, NCH, BLK], BF16, tag=f"fz{s}{b}",
                                       name=f"fz{s}{b}") for b in range(NBLK)]
                          for s in range(min(unroll, 2))]
                for u in range(unroll):
                    fz_prev = (fzsets[(u + 1) % 2] if repeat > 1 else None)
                    emit_iter(fzsets[u % 2] if repeat > 1 else fzsets[0],
                              fz_prev)

            # ---------- D: final output projection ----------
            # In the repeat loop each iteration's D is handled by the NEXT
            # iteration's pass-1 slots; only the last one remains here.
            # (For repeat=1 this is the only projection.)
            for dc in range(NCH):
                emit_D(dc, fzsets[(unroll - 1) % 2 if repeat > 1 else 0])

    nc.compile()
    _BUILD_CACHE[key] = nc
    return nc


def make_selw():
    sw = np.zeros((128, 127), np.float32)
    for p in range(128):
        sw[p, 62 + p // 64] = 1.0
    return sw


def make_selA():
    sa = np.zeros((64, NGRP, 16), np.float32)
    for r in range(M):
        by_w = {}
        for (m, w) in SRC[r]:
            by_w.setdefault(w, []).append(m)
        for (gi, w), (w2, ms) in zip(A16_GRPS[r], sorted(by_w.items())):
            assert w == w2
            for m in ms:
                for h in range(16):
                    sa[16 * m + h, gi, h] = 1.0
    return sa


def make_selB():
    # [128, NCH, 128]: row 32r+h holds, for chunk c, 0.25 * [h == 2c + j//64]
    sb = np.zeros((128, NCH, 128), np.float32)
    for r in range(M):
        for c in range(NCH):
            for j in range(128):
                sb[32 * r + 2 * c + j // 64, c, j] = 0.25
    return sb


def _vec_tile(v):
    return np.ascontiguousarray(np.asarray(v, np.float32).reshape(NCH, 128).T)


def _wf_tiles(Wf):
    """[D(out), dim(in)] fused weight -> [c, p(din%128), dk, j(dout%128)] bf16."""
    wt = np.asarray(Wf, np.float32).T                  # [din, dout]
    nk = wt.shape[0] // 128
    wt = wt.reshape(nk, 128, NCH, 128)                 # [dk, p, c, j]
    return wt.transpose(2, 1, 0, 3)                    # [c, p, dk, j]


def prepare_in_maps(inputs):
    names = [mm[0] for mm in MOD]
    emb = np.asarray(inputs["mod_emb"], np.float32)
    Wp = {pn: np.asarray(inputs[f"W{pn}"], np.float32) for pn in "qkvo"}
    bp = {pn: np.asarray(inputs[f"b{pn}"], np.float32) for pn in "qkvo"}

    shared = {}
    betaqk = np.zeros((128, M, 2, NCH), np.float32)
    betav = np.zeros((128, M, NCH), np.float32)
    tq, tk, tv = {}, {}, {}
    for i, nm in enumerate(names):
        Wm = np.asarray(inputs[f"W_{nm}"], np.float32)     # [D, dim]
        bm = np.asarray(inputs[f"b_{nm}"], np.float32) + emb[i]
        tq[i] = _wf_tiles(Wp["q"] @ Wm)
        tk[i] = _wf_tiles(Wp["k"] @ Wm)
        tv[i] = _wf_tiles(Wp["v"] @ Wm)
        betaqk[:, i, 0, :] = _vec_tile(Wp["q"] @ bm + bp["q"])
        betaqk[:, i, 1, :] = _vec_tile(Wp["k"] @ bm + bp["k"])
        betav[:, i, :] = _vec_tile(Wp["v"] @ bm + bp["v"])
    shared["betaqk"] = betaqk
    shared["betav"] = betav

    # pack all modalities along dk: [NCH, 128, 2, SNK, 128] / [NCH, 128, SNK, 128]
    wqk_all = np.concatenate(
        [np.stack([tq[m], tk[m]], axis=2) for m in range(M)], axis=3)
    shared["Wqk"] = np.ascontiguousarray(wqk_all).astype(ml_dtypes.bfloat16)
    wv_all = np.concatenate([tv[m] for m in range(M)], axis=2)
    shared["Wvf"] = np.ascontiguousarray(wv_all).astype(ml_dtypes.bfloat16)

    wo = Wp["o"].T.reshape(NCH, 128, NCH, 128)             # [dk, p, c, j]
    shared["WoT"] = np.ascontiguousarray(
        wo.transpose(2, 1, 0, 3)).astype(ml_dtypes.bfloat16)
    shared["bo"] = _vec_tile(bp["o"])
    shared["selw"] = make_selw().astype(ml_dtypes.bfloat16)
    shared["selA"] = make_selA().astype(ml_dtypes.bfloat16)
    shared["selB"] = make_selB().astype(ml_dtypes.bfloat16)

    in_maps = []
    for core in range(8):
        b, par = core // 2, core % 2
        im = dict(shared)
        for i, nm in enumerate(names):
            x = np.asarray(inputs[nm], np.float32)[b, par::2][:NLOC[i]]
            xt = x.T.reshape(NK[i], 128, NLOC[i]).transpose(1, 0, 2)
            im[f"xT{i}"] = np.ascontiguousarray(xt).astype(ml_dtypes.bfloat16)
        in_maps.append(im)
    return in_maps


def kernel(**inputs):
    inputs = {k: np.asarray(v) for k, v in inputs.items()}
    scale = float(1.0 / (np.sqrt(HD) * abs(float(inputs["temperature"]))))
    nc = build(scale, repeat=1)
    in_maps = prepare_in_maps(inputs)
    res = run_bass_kernel_spmd(nc, in_maps, list(range(8)))
    out = np.zeros((B, S, D), np.float32)
    for core in range(8):
        b, par = core // 2, core % 2
        out[b, par::2, :] = np.asarray(res.results[core]["yT"],
                                       np.float32).T
    return out


# revision 30
# speedup vs baseline: 1.2605x; 1.0014x over previous
"""Trainium2 Bass kernel for nn_CantorModalityFusion.

Sharding: 8 cores = (batch b in 0..3) x (position parity in 0..1).
Each core handles batch b, positions s = par, par+2, ... (1024 positions).
The computation is per-(b, s) independent -> no collectives.

The per-modality input projection is folded into the QKV weights on the
host (Wf = Wp @ W_m, beta = Wp @ (b_m + emb_m) + b_p), so the device
computes q/k/v for each modality directly from the raw modality input
(contraction over dim_m instead of D). Weights and x stream in bf16
(fp32 PSUM accumulation); q/k/v, scores, softmax, fused accumulation
stay fp32/bf16 mixed; output is written bf16.

v3 layout: chunk-outer loops process BOTH 512-position blocks under one
weight load, so each weight byte is fetched once per iteration (~35MB
instead of ~66MB of HBM traffic per core). The repeat loop (timing) uses
staggered_reset so iterations overlap point-to-point instead of through
an all-engine barrier.

Per iteration:
  P1: for c: q.T/k.T chains for both blocks from x; scores accumulate
      into 3 pinned PSUM banks (block0 in partitions 0-63, block1 in
      64-127 via col tile_position)                              [PE+DVE]
  SM: softmax per block, DVE reading scores straight from PSUM  [DVE+ACT]
  A16: per-source summed attn, 4 sources packed into one PSUM bank at
      partition offsets {0,32,64,96} via col tile_position       [PE]
  P2: for c: v.T chains both blocks; Abc = bcast(A16) via row-packed
      16-contraction matmuls; fused.T[c] = sum_r Abc_r * v.T[r] [PE+DVE]
  D:  y.T = Wo.T.T @ fused.T (+ bo) per 128-feature chunk        [PE+ACT]
"""

import os
import sys

import numpy as np

sys.path.insert(0, "/opt/trn_rl_repo")

import ml_dtypes

import concourse.bacc as bacc
import concourse.mybir as mybir
from concourse import tile
from concourse.bass_utils import run_bass_kernel_spmd

F32 = mybir.dt.float32
F32R = mybir.dt.float32r
BF16 = mybir.dt.bfloat16
AF = mybir.ActivationFunctionType
ALU = mybir.AluOpType

B, S, D, H, HD = 4, 2048, 1024, 16, 64
M, WIN = 4, 3
MOD = [("text", 768, 2048), ("image", 1024, 1024), ("audio", 512, 1500), ("video", 2048, 512)]
ROUTES = [[0, 1, 2], [0, 1, 2], [2, 3, 0], [3, 2, 0]]
PAIRS = [(m, w, ROUTES[m][w]) for m in range(M) for w in range(WIN)]
SRC = {r: [(m, w) for (m, w, rr) in PAIRS if rr == r] for r in range(M)}
PAIR_IDX = {(m, w): m * WIN + w for m in range(M) for w in range(WIN)}

NPOS = S // 2
BLK = 512
NBLK = NPOS // BLK
NCH = D // 128                           # 8 output feature chunks
NLOC = [sl // 2 for (_, _, sl) in MOD]   # 1024, 512, 750, 256
NK = [dim // 128 for (_, dim, _) in MOD]  # 6, 8, 4, 16 input chunks
SNK = sum(NK)                             # 34
KOFF = [sum(NK[:m]) for m in range(M)]    # dk offset of modality m

_BUILD_CACHE = {}
_PROBE = os.environ.get("PROBE", "")


def n_active(m, blk):
    return max(0, min(BLK, NLOC[m] - blk * BLK))


def build(scale, repeat=1):
    key = (float(scale), repeat)
    if key in _BUILD_CACHE:
        return _BUILD_CACHE[key]
    nc = bacc.Bacc("TRN2", target_bir_lowering=False, debug=False)

    # x pre-permuted on the host: [p(din%128), dk, pos] -> one DMA per m
    xT = [nc.dram_tensor(f"xT{m}", [128, NK[m], NLOC[m]], BF16,
                         kind="ExternalInput") for m in range(M)]
    # fused q/k weights: [c, p(din%128), ti(q/k), dk, j(dout%128)]
    # all modalities' weights packed along dk: one DMA per (chunk, q/k|v)
    Wqk = nc.dram_tensor("Wqk", [NCH, 128, 2, SNK, 128], BF16,
                         kind="ExternalInput")
    Wvf = nc.dram_tensor("Wvf", [NCH, 128, SNK, 128], BF16,
                         kind="ExternalInput")
    WoT = nc.dram_tensor("WoT", [NCH, 128, NCH, 128], BF16, kind="ExternalInput")
    betaqk_d = nc.dram_tensor("betaqk", [128, M, 2, NCH], F32, kind="ExternalInput")
    betav_d = nc.dram_tensor("betav", [128, M, NCH], F32, kind="ExternalInput")
    bo_d = nc.dram_tensor("bo", [128, NCH], F32, kind="ExternalInput")
    selw_d = nc.dram_tensor("selw", [128, 127], BF16, kind="ExternalInput")
    selA_d = nc.dram_tensor("selA", [64, M * WIN, 16], BF16, kind="ExternalInput")
    selB_d = nc.dram_tensor("selB", [128, NCH, 128], BF16, kind="ExternalInput")
    yT = nc.dram_tensor("yT", [D, NPOS], BF16, kind="ExternalOutput")

    # per-block active modalities / pairs
    nact = [[n_active(m, b) for m in range(M)] for b in range(NBLK)]
    act_m = [[m for m in range(M) if nact[b][m] > 0] for b in range(NBLK)]
    act_pairs = [[(m, w, r) for (m, w, r) in PAIRS
                  if nact[b][m] > 0 and nact[b][r] > 0] for b in range(NBLK)]
    n_sc = {(w, b): sum(1 for (m, w2, r) in act_pairs[b] if w2 == w) * NCH
            for w in range(WIN) for b in range(NBLK)}

    unroll = 4 if repeat % 4 == 0 else (2 if repeat > 1 else 1)
    assert repeat % unroll == 0
    trips = repeat // unroll

    with tile.TileContext(nc) as tc:
        with (
            tc.tile_pool(name="const", bufs=1) as cpool,
            tc.tile_pool(name="wq", bufs=2) as wqpool,
            tc.tile_pool(name="wo", bufs=2) as wopool,
            tc.tile_pool(name="xt", bufs=1) as xtpool,
            tc.tile_pool(name="qk", bufs=1) as qkpool,
            tc.tile_pool(name="pr", bufs=2) as prpool,
            tc.tile_pool(name="sm", bufs=1) as smpool,
            tc.tile_pool(name="fz", bufs=1) as fzpool,
            tc.tile_pool(name="yo", bufs=2) as yopool,
            tc.tile_pool(name="ps", bufs=1, space="PSUM") as pspool,
        ):
            def psum(i):
                return pspool.tile([128, BLK], F32, tag=f"a{i}", name=f"ps_a{i}")

            # ---- constants ----
            selw = cpool.tile([128, 127], BF16, tag="selw")
            nc.sync.dma_start(selw[:], selw_d[:])
            selA = cpool.tile([64, M * WIN, 16], BF16, tag="selA")
            nc.sync.dma_start(selA[:], selA_d[:])
            selB = cpool.tile([128, NCH, 128], BF16, tag="selB")
            nc.sync.dma_start(selB[:], selB_d[:])
            betaqk = cpool.tile([128, M, 2, NCH], F32, tag="betaqk")
            nc.sync.dma_start(betaqk[:], betaqk_d[:])
            betav = cpool.tile([128, M, NCH], F32, tag="betav")
            nc.sync.dma_start(betav[:], betav_d[:])
            bo = cpool.tile([128, NCH], F32, tag="bo")
            nc.sync.dma_start(bo[:], bo_d[:])

            qk_ctr = [0]
            pr_ctr = [0]

            def emit_D(dc, fz_tiles):
                if _PROBE == "qkv":
                    return
                wsl = wopool.tile([128, NCH, 128], BF16,
                                  tag="wo", name="wsld")
                nc.sync.dma_start(wsl[:], WoT[dc])
                for b in range(NBLK):
                    acc = psum(6 + b)
                    for dk in range(NCH):
                        nc.tensor.matmul(
                            acc[:], wsl[:, dk, :], fz_tiles[b][:, dk, :],
                            start=(dk == 0), stop=(dk == NCH - 1),
                            skip_group_check=True)
                    yo = yopool.tile([128, BLK], BF16, tag="yo")
                    nc.vector.tensor_scalar_add(yo[:], acc[:],
                                                bo[:, dc:dc + 1])
                    nc.gpsimd.dma_start(
                        yT[dc * 128:(dc + 1) * 128,
                           b * BLK:(b + 1) * BLK], yo[:])

            def emit_iter(fz_this, fz_prev):
                """One full iteration. fz_prev (if set) is the previous
                iteration's fused output: its projection is interleaved into
                pass 1 so its PE work covers the x/weight reload window."""
                # x rides the gpsimd (SWDGE) queue so its cross-iteration WAR
                # waits never block the weight stream on the sync HWDGE ring.
                xt = {}
                for m in range(M):
                    t = xtpool.tile([128, NK[m], NLOC[m]], BF16,
                                    tag=f"x{m}", name="xtile")
                    nc.gpsimd.dma_start(t[:], xT[m][:])
                    xt[m] = t

                # ---------- pass 1: q, k, scores ----------
                # scores psum: tag a3+w, block0 rows 0-63, block1 rows 64-127
                sc_ps = [psum(3 + w) for w in range(WIN)]
                c_sc = {(w, b): 0 for w in range(WIN) for b in range(NBLK)}

                def emit_qk(c):
                    qk_t = {}
                    wqk = wqpool.tile([128, 2, SNK, 128], BF16,
                                      tag="wqk", name="wqk")
                    for m in range(M):
                        nc.sync.dma_start(
                            wqk[:, :, KOFF[m]:KOFF[m] + NK[m], :],
                            Wqk[c][:, :, KOFF[m]:KOFF[m] + NK[m], :])
                    for m in range(M):
                        for ti, tname in enumerate("qk"):
                            for b in range(NBLK):
                                na = nact[b][m]
                                if na == 0:
                                    continue
                                p0 = b * BLK
                                acc = psum(qk_ctr[0] % 3)
                                qk_ctr[0] += 1
                                for dk in range(NK[m]):
                                    nc.tensor.matmul(
                                        acc[:, :na],
                                        wqk[:, ti, KOFF[m] + dk, :],
                                        xt[m][:, dk, p0:p0 + na],
                                        start=(dk == 0), stop=(dk == NK[m] - 1),
                                        skip_group_check=True)
                                t = qkpool.tile([128, BLK], BF16,
                                                tag=f"{tname}{m}{b}_{c % 2}",
                                                name=f"{tname}{m}{b}")
                                nc.scalar.activation(
                                    t[:, :na], acc[:, :na], AF.Identity,
                                    bias=betaqk[:, m, ti, c:c + 1])
                                if na < BLK:
                                    nc.gpsimd.memset(t[:, na:].bitcast(F32), 0.0)
                                qk_t[(tname, m, b)] = t
                    return qk_t

                def emit_scores(c, qk_t):
                    if _PROBE == "qkv":
                        return
                    for b in range(NBLK):
                        for (m, w, r) in act_pairs[b]:
                            i = c_sc[(w, b)]
                            c_sc[(w, b)] += 1
                            # first matmul of a window must cover the full
                            # width (start=True zeroes the tail); later ones
                            # only where q*k can be nonzero
                            nw = BLK if i == 0 else min(nact[b][m], nact[b][r])
                            prod = prpool.tile([128, BLK], BF16, bufs=1,
                                               tag=f"prod{pr_ctr[0] % 6}",
                                               name="prod")
                            pr_ctr[0] += 1
                            nc.vector.tensor_mul(
                                prod[:, :nw], qk_t[("q", m, b)][:, :nw],
                                qk_t[("k", r, b)][:, :nw])
                            off = 62 - (16 * m + 2 * c)
                            nc.tensor.matmul(
                                sc_ps[w][64 * b:64 * b + 64, :nw],
                                selw[:, off:off + 64],
                                prod[:, :nw],
                                start=(i == 0), stop=(i == n_sc[(w, b)] - 1),
                                skip_group_check=True)

                prev = emit_qk(0)
                for c in range(1, NCH):
                    cur = emit_qk(c)
                    emit_scores(c - 1, prev)
                    if fz_prev is not None:
                        emit_D(c - 1, fz_prev)
                    prev = cur
                emit_scores(NCH - 1, prev)
                if fz_prev is not None:
                    emit_D(NCH - 1, fz_prev)

                # ---------- softmax (per block; DVE reads scores PSUM) ----
                attn = {}
                for b in (() if _PROBE == "qkv" else range(NBLK)):
                    act_w = [w for w in range(WIN) if n_sc[(w, b)] > 0]
                    sl = slice(64 * b, 64 * b + 64)
                    # DVE may read only one PSUM operand per instruction:
                    # copy the first window out, then chain maxes
                    mx = smpool.tile([64, BLK], F32, tag=f"mx{b}")
                    nc.vector.tensor_copy(mx[:], sc_ps[act_w[0]][sl, :])
                    for w in act_w[1:]:
                        nc.vector.tensor_tensor(
                            mx[:], mx[:], sc_ps[w][sl, :], op=ALU.max)
                    if len(act_w) < WIN:
                        # empty windows score 0: it participates in the max
                        nc.vector.tensor_scalar_max(mx[:], mx[:], 0.0)
                    for w in range(WIN):
                        a = smpool.tile([64, BLK], BF16, tag=f"at{w}{b}",
                                        name="attn")
                        if w in act_w:
                            ssub = smpool.tile([64, BLK], F32, bufs=2,
                                               tag="ss", name="ssub")
                            nc.vector.tensor_tensor(
                                ssub[:], sc_ps[w][sl, :], mx[:],
                                op=ALU.subtract)
                            nc.scalar.activation(a[:], ssub[:], AF.Exp,
                                                 scale=scale)
                        else:
                            nc.scalar.activation(a[:], mx[:], AF.Exp,
                                                 scale=-scale)
                        attn[(w, b)] = a
                    den = smpool.tile([64, BLK], F32, tag=f"mx{b}", name="den")
                    nc.vector.tensor_add(den[:], attn[(0, b)][:],
                                         attn[(1, b)][:])
                    nc.vector.tensor_add(den[:], den[:], attn[(2, b)][:])
                    rec = smpool.tile([64, BLK], F32R, tag=f"rec{b}")
                    with nc.allow_low_precision(reason="fp32r attn weights"):
                        nc.vector.reciprocal(rec[:], den[:])
                    for w in range(WIN):
                        nc.vector.tensor_mul(attn[(w, b)][:], attn[(w, b)][:],
                                             rec[:])

                # ---------- A16: per-source summed attn, packed in one bank
                # at partition offsets 32r (col tile_position) ----------
                a16sb = {}
                for b in (() if _PROBE == "qkv" else range(NBLK)):
                    a16 = psum(5)
                    for r in range(M):
                        if nact[b][r] == 0:
                            continue
                        srcs = SRC[r]
                        for i, (m, w) in enumerate(srcs):
                            nc.tensor.matmul(
                                a16[32 * r:32 * r + 16, :],
                                selA[:, PAIR_IDX[(m, w)], :],
                                attn[(w, b)][:],
                                start=(i == 0), stop=(i == len(srcs) - 1),
                                skip_group_check=True,
                                tile_position=(0, 32 * r))
                    t = smpool.tile([128, BLK], BF16, tag=f"a16_{b}",
                                    name="a16sb")
                    nc.scalar.activation(t[:], a16[:], AF.Identity)
                    a16sb[b] = t

                # ---------- pass 2: v, Abc, fused ----------
                def emit_v(c):
                    v_t = {}
                    wv = wqpool.tile([128, SNK, 128], BF16,
                                     tag="wv", name="wv")
                    for m in range(M):
                        nc.sync.dma_start(
                            wv[:, KOFF[m]:KOFF[m] + NK[m], :],
                            Wvf[c][:, KOFF[m]:KOFF[m] + NK[m], :])
                    for m in range(M):
                        for b in range(NBLK):
                            na = nact[b][m]
                            if na == 0:
                                continue
                            p0 = b * BLK
                            acc = v_ctr_rot()
                            for dk in range(NK[m]):
                                nc.tensor.matmul(
                                    acc[:, :na], wv[:, KOFF[m] + dk, :],
                                    xt[m][:, dk, p0:p0 + na],
                                    start=(dk == 0), stop=(dk == NK[m] - 1),
                                    skip_group_check=True)
                            t = qkpool.tile([128, BLK], BF16,
                                            tag=f"q{m}{b}_{c % 2}",
                                            name="vt")
                            nc.vector.tensor_scalar_add(
                                t[:, :na], acc[:, :na],
                                betav[:, m, c:c + 1])
                            if na < BLK:
                                nc.gpsimd.memset(t[:, na:].bitcast(F32), 0.0)
                            v_t[(m, b)] = t
                    return v_t

                VROT = [0, 1, 2, 6, 7]
                def v_ctr_rot():
                    i = VROT[qk_ctr[0] % 5]
                    qk_ctr[0] += 1
                    return psum(i)

                ab_ctr = [0]

                def emit_fused(c, v_t):
                    if _PROBE == "qkv":
                        return
                    for b in range(NBLK):
                        rs = act_m[b]
                        # row-packed broadcast matmuls, two at a time
                        ab_ps = {}
                        for ri in range(0, len(rs), 2):
                            for j, r in enumerate(rs[ri:ri + 2]):
                                ab = psum(3 + (ab_ctr[0] % 3))
                                ab_ctr[0] += 1
                                nc.tensor.matmul(
                                    ab[:], selB[32 * r:32 * r + 16, c, :],
                                    a16sb[b][32 * r:32 * r + 16, :],
                                    start=True, stop=True,
                                    skip_group_check=True,
                                    tile_position=(32 * r, 0))
                                ab_ps[r] = ab
                        r0 = rs[0]
                        accv = prpool.tile([128, BLK], F32, bufs=1,
                                           tag="f0", name="accv")
                        nc.vector.tensor_mul(accv[:], ab_ps[r0][:],
                                             v_t[(r0, b)][:])
                        if len(rs) == 1:
                            nc.vector.tensor_copy(fz_this[b][:, c, :], accv[:])
                        for j, r in enumerate(rs[1:]):
                            tmp = prpool.tile([128, BLK], F32, bufs=1,
                                              tag="f1", name="tmp")
                            nc.vector.tensor_mul(tmp[:], ab_ps[r][:],
                                                 v_t[(r, b)][:])
                            last = (j == len(rs) - 2)
                            nc.vector.tensor_add(
                                fz_this[b][:, c, :] if last else accv[:],
                                accv[:], tmp[:])

                vbuf = {0: emit_v(0), 1: emit_v(1)}
                for c in range(NCH):
                    emit_fused(c, vbuf.pop(c))
                    if c + 2 < NCH:
                        vbuf[c + 2] = emit_v(c + 2)

            import contextlib
            rep_cm = (tc.For_i(0, trips, 1,
                               hint_engines=(mybir.EngineType.PE,
                                             mybir.EngineType.Activation,
                                             mybir.EngineType.DVE,
                                             mybir.EngineType.SP,
                                             mybir.EngineType.Pool),
                               staggered_reset=True)
                      if trips > 1 else contextlib.nullcontext())

            with rep_cm:
                # two fused-output buffer sets, alternating per unroll; the
                # D projection of set s is interleaved into the NEXT
                # iteration's pass 1 (cross-trip for the last unroll)
                fzsets = [[fzpool.tile([128, NCH, BLK], BF16, tag=f"fz{s}{b}",
                                       name=f"fz{s}{b}") for b in range(NBLK)]
                          for s in range(min(unroll, 2))]
                for u in range(unroll):
                    fz_prev = (fzsets[(u + 1) % 2] if repeat > 1 else None)
                    emit_iter(fzsets[u % 2] if repeat > 1 else fzsets[0],
                              fz_prev)

            # ---------- D: final output projection ----------
            # In the repeat loop each iteration's D is handled by the NEXT
            # iteration's pass-1 slots; only the last one remains here.
            # (For repeat=1 this is the only projection.)
            for dc in range(NCH):
                emit_D(dc, fzsets[(unroll - 1) % 2 if repeat > 1 else 0])

    nc.compile()
    _BUILD_CACHE[key] = nc
    return nc


def make_selw():
    sw = np.zeros((128, 127), np.float32)
    for p in range(128):
        sw[p, 62 + p // 64] = 1.0
    return sw


def make_selA():
    sa = np.zeros((64, M * WIN, 16), np.float32)
    for m in range(M):
        for w in range(WIN):
            for h in range(16):
                sa[16 * m + h, m * WIN + w, h] = 1.0
    return sa


def make_selB():
    # [128, NCH, 128]: row 32r+h holds, for chunk c, 0.25 * [h == 2c + j//64]
    sb = np.zeros((128, NCH, 128), np.float32)
    for r in range(M):
        for c in range(NCH):
            for j in range(128):
                sb[32 * r + 2 * c + j // 64, c, j] = 0.25
    return sb


def _vec_tile(v):
    return np.ascontiguousarray(np.asarray(v, np.float32).reshape(NCH, 128).T)


def _wf_tiles(Wf):
    """[D(out), dim(in)] fused weight -> [c, p(din%128), dk, j(dout%128)] bf16."""
    wt = np.asarray(Wf, np.float32).T                  # [din, dout]
    nk = wt.shape[0] // 128
    wt = wt.reshape(nk, 128, NCH, 128)                 # [dk, p, c, j]
    return wt.transpose(2, 1, 0, 3)                    # [c, p, dk, j]


def prepare_in_maps(inputs):
    names = [mm[0] for mm in MOD]
    emb = np.asarray(inputs["mod_emb"], np.float32)
    Wp = {pn: np.asarray(inputs[f"W{pn}"], np.float32) for pn in "qkvo"}
    bp = {pn: np.asarray(inputs[f"b{pn}"], np.float32) for pn in "qkvo"}

    shared = {}
    betaqk = np.zeros((128, M, 2, NCH), np.float32)
    betav = np.zeros((128, M, NCH), np.float32)
    tq, tk, tv = {}, {}, {}
    for i, nm in enumerate(names):
        Wm = np.asarray(inputs[f"W_{nm}"], np.float32)     # [D, dim]
        bm = np.asarray(inputs[f"b_{nm}"], np.float32) + emb[i]
        tq[i] = _wf_tiles(Wp["q"] @ Wm)
        tk[i] = _wf_tiles(Wp["k"] @ Wm)
        tv[i] = _wf_tiles(Wp["v"] @ Wm)
        betaqk[:, i, 0, :] = _vec_tile(Wp["q"] @ bm + bp["q"])
        betaqk[:, i, 1, :] = _vec_tile(Wp["k"] @ bm + bp["k"])
        betav[:, i, :] = _vec_tile(Wp["v"] @ bm + bp["v"])
    shared["betaqk"] = betaqk
    shared["betav"] = betav

    # pack all modalities along dk: [NCH, 128, 2, SNK, 128] / [NCH, 128, SNK, 128]
    wqk_all = np.concatenate(
        [np.stack([tq[m], tk[m]], axis=2) for m in range(M)], axis=3)
    shared["Wqk"] = np.ascontiguousarray(wqk_all).astype(ml_dtypes.bfloat16)
    wv_all = np.concatenate([tv[m] for m in range(M)], axis=2)
    shared["Wvf"] = np.ascontiguousarray(wv_all).astype(ml_dtypes.bfloat16)

    wo = Wp["o"].T.reshape(NCH, 128, NCH, 128)             # [dk, p, c, j]
    shared["WoT"] = np.ascontiguousarray(
        wo.transpose(2, 1, 0, 3)).astype(ml_dtypes.bfloat16)
    shared["bo"] = _vec_tile(bp["o"])
    shared["selw"] = make_selw().astype(ml_dtypes.bfloat16)
    shared["selA"] = make_selA().astype(ml_dtypes.bfloat16)
    shared["selB"] = make_selB().astype(ml_dtypes.bfloat16)

    in_maps = []
    for core in range(8):
        b, par = core // 2, core % 2
        im = dict(shared)
        for i, nm in enumerate(names):
            x = np.asarray(inputs[nm], np.float32)[b, par::2][:NLOC[i]]
            xt = x.T.reshape(NK[i], 128, NLOC[i]).transpose(1, 0, 2)
            im[f"xT{i}"] = np.ascontiguousarray(xt).astype(ml_dtypes.bfloat16)
        in_maps.append(im)
    return in_maps


def kernel(**inputs):
    inputs = {k: np.asarray(v) for k, v in inputs.items()}
    scale = float(1.0 / (np.sqrt(HD) * abs(float(inputs["temperature"]))))
    nc = build(scale, repeat=1)
    in_maps = prepare_in_maps(inputs)
    res = run_bass_kernel_spmd(nc, in_maps, list(range(8)))
    out = np.zeros((B, S, D), np.float32)
    for core in range(8):
        b, par = core // 2, core % 2
        out[b, par::2, :] = np.asarray(res.results[core]["yT"],
                                       np.float32).T
    return out


# revision 31
# speedup vs baseline: 1.2689x; 1.0067x over previous
"""Trainium2 Bass kernel for nn_CantorModalityFusion.

Sharding: 8 cores = (batch b in 0..3) x (position parity in 0..1).
Each core handles batch b, positions s = par, par+2, ... (1024 positions).
The computation is per-(b, s) independent -> no collectives.

The per-modality input projection is folded into the QKV weights on the
host (Wf = Wp @ W_m, beta = Wp @ (b_m + emb_m) + b_p), so the device
computes q/k/v for each modality directly from the raw modality input
(contraction over dim_m instead of D). Weights and x stream in bf16
(fp32 PSUM accumulation); q/k/v, scores, softmax, fused accumulation
stay fp32/bf16 mixed; output is written bf16.

v3 layout: chunk-outer loops process BOTH 512-position blocks under one
weight load, so each weight byte is fetched once per iteration (~35MB
instead of ~66MB of HBM traffic per core). The repeat loop (timing) uses
staggered_reset so iterations overlap point-to-point instead of through
an all-engine barrier.

Per iteration:
  P1: for c: q.T/k.T chains for both blocks from x; scores accumulate
      into 3 pinned PSUM banks (block0 in partitions 0-63, block1 in
      64-127 via col tile_position)                              [PE+DVE]
  SM: softmax per block, DVE reading scores straight from PSUM  [DVE+ACT]
  A16: per-source summed attn, 4 sources packed into one PSUM bank at
      partition offsets {0,32,64,96} via col tile_position       [PE]
  P2: for c: v.T chains both blocks; Abc = bcast(A16) via row-packed
      16-contraction matmuls; fused.T[c] = sum_r Abc_r * v.T[r] [PE+DVE]
  D:  y.T = Wo.T.T @ fused.T (+ bo) per 128-feature chunk        [PE+ACT]
"""

import os
import sys

import numpy as np

sys.path.insert(0, "/opt/trn_rl_repo")

import ml_dtypes

import concourse.bacc as bacc
import concourse.mybir as mybir
from concourse import tile
from concourse.bass_utils import run_bass_kernel_spmd

F32 = mybir.dt.float32
F32R = mybir.dt.float32r
BF16 = mybir.dt.bfloat16
AF = mybir.ActivationFunctionType
ALU = mybir.AluOpType

B, S, D, H, HD = 4, 2048, 1024, 16, 64
M, WIN = 4, 3
MOD = [("text", 768, 2048), ("image", 1024, 1024), ("audio", 512, 1500), ("video", 2048, 512)]
ROUTES = [[0, 1, 2], [0, 1, 2], [2, 3, 0], [3, 2, 0]]
PAIRS = [(m, w, ROUTES[m][w]) for m in range(M) for w in range(WIN)]
SRC = {r: [(m, w) for (m, w, rr) in PAIRS if rr == r] for r in range(M)}
PAIR_IDX = {(m, w): m * WIN + w for m in range(M) for w in range(WIN)}

NPOS = S // 2
BLK = 512
NBLK = NPOS // BLK
NCH = D // 128                           # 8 output feature chunks
NLOC = [sl // 2 for (_, _, sl) in MOD]   # 1024, 512, 750, 256
NK = [dim // 128 for (_, dim, _) in MOD]  # 6, 8, 4, 16 input chunks
SNK = sum(NK)                             # 34
KOFF = [sum(NK[:m]) for m in range(M)]    # dk offset of modality m

_BUILD_CACHE = {}
_PROBE = os.environ.get("PROBE", "")


def n_active(m, blk):
    return max(0, min(BLK, NLOC[m] - blk * BLK))


def build(scale, repeat=1):
    key = (float(scale), repeat)
    if key in _BUILD_CACHE:
        return _BUILD_CACHE[key]
    nc = bacc.Bacc("TRN2", target_bir_lowering=False, debug=False)

    # x pre-permuted on the host: [p(din%128), dk, pos] -> one DMA per m
    xT = [nc.dram_tensor(f"xT{m}", [128, NK[m], NLOC[m]], BF16,
                         kind="ExternalInput") for m in range(M)]
    # fused q/k weights: [c, p(din%128), ti(q/k), dk, j(dout%128)]
    # all modalities' weights packed along dk: one DMA per (chunk, q/k|v)
    Wqk = nc.dram_tensor("Wqk", [NCH, 128, 2, SNK, 128], BF16,
                         kind="ExternalInput")
    Wvf = nc.dram_tensor("Wvf", [NCH, 128, SNK, 128], BF16,
                         kind="ExternalInput")
    WoT = nc.dram_tensor("WoT", [NCH, 128, NCH, 128], BF16, kind="ExternalInput")
    betaqk_d = nc.dram_tensor("betaqk", [128, M, 2, NCH], F32, kind="ExternalInput")
    betav_d = nc.dram_tensor("betav", [128, M, NCH], F32, kind="ExternalInput")
    bo_d = nc.dram_tensor("bo", [128, NCH], F32, kind="ExternalInput")
    selw_d = nc.dram_tensor("selw", [128, 127], BF16, kind="ExternalInput")
    selA_d = nc.dram_tensor("selA", [64, M * WIN, 16], BF16, kind="ExternalInput")
    selB_d = nc.dram_tensor("selB", [128, NCH, 128], BF16, kind="ExternalInput")
    yT = nc.dram_tensor("yT", [D, NPOS], BF16, kind="ExternalOutput")

    # per-block active modalities / pairs
    nact = [[n_active(m, b) for m in range(M)] for b in range(NBLK)]
    act_m = [[m for m in range(M) if nact[b][m] > 0] for b in range(NBLK)]
    act_pairs = [[(m, w, r) for (m, w, r) in PAIRS
                  if nact[b][m] > 0 and nact[b][r] > 0] for b in range(NBLK)]
    n_sc = {(w, b): sum(1 for (m, w2, r) in act_pairs[b] if w2 == w) * NCH
            for w in range(WIN) for b in range(NBLK)}

    unroll = 4 if repeat % 4 == 0 else (2 if repeat > 1 else 1)
    assert repeat % unroll == 0
    trips = repeat // unroll

    with tile.TileContext(nc) as tc:
        with (
            tc.tile_pool(name="const", bufs=1) as cpool,
            tc.tile_pool(name="wq", bufs=2) as wqpool,
            tc.tile_pool(name="wo", bufs=2) as wopool,
            tc.tile_pool(name="xt", bufs=1) as xtpool,
            tc.tile_pool(name="qk", bufs=1) as qkpool,
            tc.tile_pool(name="pr", bufs=2) as prpool,
            tc.tile_pool(name="sm", bufs=1) as smpool,
            tc.tile_pool(name="fz", bufs=1) as fzpool,
            tc.tile_pool(name="yo", bufs=2) as yopool,
            tc.tile_pool(name="ps", bufs=1, space="PSUM") as pspool,
        ):
            def psum(i):
                return pspool.tile([128, BLK], F32, tag=f"a{i}", name=f"ps_a{i}")

            # ---- constants ----
            selw = cpool.tile([128, 127], BF16, tag="selw")
            nc.sync.dma_start(selw[:], selw_d[:])
            selA = cpool.tile([64, M * WIN, 16], BF16, tag="selA")
            nc.sync.dma_start(selA[:], selA_d[:])
            selB = cpool.tile([128, NCH, 128], BF16, tag="selB")
            nc.sync.dma_start(selB[:], selB_d[:])
            betaqk = cpool.tile([128, M, 2, NCH], F32, tag="betaqk")
            nc.sync.dma_start(betaqk[:], betaqk_d[:])
            betav = cpool.tile([128, M, NCH], F32, tag="betav")
            nc.sync.dma_start(betav[:], betav_d[:])
            bo = cpool.tile([128, NCH], F32, tag="bo")
            nc.sync.dma_start(bo[:], bo_d[:])

            qk_ctr = [0]
            pr_ctr = [0]

            def emit_D(dc, fz_tiles):
                if _PROBE == "qkv":
                    return
                wsl = wopool.tile([128, NCH, 128], BF16,
                                  tag="wo", name="wsld")
                nc.sync.dma_start(wsl[:], WoT[dc])
                for b in range(NBLK):
                    acc = psum(6 + b)
                    for dk in range(NCH):
                        nc.tensor.matmul(
                            acc[:], wsl[:, dk, :], fz_tiles[b][:, dk, :],
                            start=(dk == 0), stop=(dk == NCH - 1),
                            skip_group_check=True)
                    yo = yopool.tile([128, BLK], BF16, tag="yo")
                    nc.vector.tensor_scalar_add(yo[:], acc[:],
                                                bo[:, dc:dc + 1])
                    nc.gpsimd.dma_start(
                        yT[dc * 128:(dc + 1) * 128,
                           b * BLK:(b + 1) * BLK], yo[:])

            def emit_iter(fz_this, fz_prev):
                """One full iteration. fz_prev (if set) is the previous
                iteration's fused output: its projection is interleaved into
                pass 1 so its PE work covers the x/weight reload window."""
                # x rides the gpsimd (SWDGE) queue so its cross-iteration WAR
                # waits never block the weight stream on the sync HWDGE ring.
                xt = {}
                for m in range(M):
                    t = xtpool.tile([128, NK[m], NLOC[m]], BF16,
                                    tag=f"x{m}", name="xtile")
                    nc.gpsimd.dma_start(t[:], xT[m][:])
                    xt[m] = t

                # ---------- pass 1: q, k, scores ----------
                # scores psum: tag a3+w, block0 rows 0-63, block1 rows 64-127
                sc_ps = [psum(3 + w) for w in range(WIN)]
                c_sc = {(w, b): 0 for w in range(WIN) for b in range(NBLK)}

                def emit_qk(c):
                    qk_t = {}
                    wqk = wqpool.tile([128, 2, SNK, 128], BF16,
                                      tag="wqk", name="wqk")
                    for m in range(M):
                        nc.sync.dma_start(
                            wqk[:, :, KOFF[m]:KOFF[m] + NK[m], :],
                            Wqk[c][:, :, KOFF[m]:KOFF[m] + NK[m], :])
                    for m in range(M):
                        for ti, tname in enumerate("qk"):
                            for b in range(NBLK):
                                na = nact[b][m]
                                if na == 0:
                                    continue
                                p0 = b * BLK
                                acc = psum(qk_ctr[0] % 3)
                                qk_ctr[0] += 1
                                for dk in range(NK[m]):
                                    nc.tensor.matmul(
                                        acc[:, :na],
                                        wqk[:, ti, KOFF[m] + dk, :],
                                        xt[m][:, dk, p0:p0 + na],
                                        start=(dk == 0), stop=(dk == NK[m] - 1),
                                        skip_group_check=True)
                                t = qkpool.tile([128, BLK], BF16,
                                                tag=f"{tname}{m}{b}_{c % 2}",
                                                name=f"{tname}{m}{b}")
                                nc.scalar.activation(
                                    t[:, :na], acc[:, :na], AF.Identity,
                                    bias=betaqk[:, m, ti, c:c + 1])
                                if na < BLK:
                                    nc.gpsimd.memset(t[:, na:].bitcast(F32), 0.0)
                                qk_t[(tname, m, b)] = t
                    return qk_t

                def emit_scores(c, qk_t):
                    if _PROBE == "qkv":
                        return
                    for b in range(NBLK):
                        for (m, w, r) in act_pairs[b]:
                            i = c_sc[(w, b)]
                            c_sc[(w, b)] += 1
                            # first matmul of a window must cover the full
                            # width (start=True zeroes the tail); later ones
                            # only where q*k can be nonzero
                            nw = BLK if i == 0 else min(nact[b][m], nact[b][r])
                            prod = prpool.tile([128, BLK], BF16, bufs=1,
                                               tag=f"prod{pr_ctr[0] % 6}",
                                               name="prod")
                            pr_ctr[0] += 1
                            nc.vector.tensor_mul(
                                prod[:, :nw], qk_t[("q", m, b)][:, :nw],
                                qk_t[("k", r, b)][:, :nw])
                            off = 62 - (16 * m + 2 * c)
                            nc.tensor.matmul(
                                sc_ps[w][64 * b:64 * b + 64, :nw],
                                selw[:, off:off + 64],
                                prod[:, :nw],
                                start=(i == 0), stop=(i == n_sc[(w, b)] - 1),
                                skip_group_check=True)

                prev = emit_qk(0)
                for c in range(1, NCH):
                    cur = emit_qk(c)
                    emit_scores(c - 1, prev)
                    if fz_prev is not None:
                        emit_D(c - 1, fz_prev)
                    prev = cur
                emit_scores(NCH - 1, prev)
                if fz_prev is not None:
                    emit_D(NCH - 1, fz_prev)

                # ---------- pass 2: v, Abc, fused ----------
                def emit_v(c):
                    v_t = {}
                    wv = wqpool.tile([128, SNK, 128], BF16,
                                     tag="wv", name="wv")
                    for m in range(M):
                        nc.sync.dma_start(
                            wv[:, KOFF[m]:KOFF[m] + NK[m], :],
                            Wvf[c][:, KOFF[m]:KOFF[m] + NK[m], :])
                    for m in range(M):
                        for b in range(NBLK):
                            na = nact[b][m]
                            if na == 0:
                                continue
                            p0 = b * BLK
                            acc = v_ctr_rot()
                            for dk in range(NK[m]):
                                nc.tensor.matmul(
                                    acc[:, :na], wv[:, KOFF[m] + dk, :],
                                    xt[m][:, dk, p0:p0 + na],
                                    start=(dk == 0), stop=(dk == NK[m] - 1),
                                    skip_group_check=True)
                            t = qkpool.tile([128, BLK], BF16,
                                            tag=f"q{m}{b}_{c % 2}",
                                            name="vt")
                            nc.vector.tensor_scalar_add(
                                t[:, :na], acc[:, :na],
                                betav[:, m, c:c + 1])
                            if na < BLK:
                                nc.gpsimd.memset(t[:, na:].bitcast(F32), 0.0)
                            v_t[(m, b)] = t
                    return v_t

                VROT = [0, 1, 2, 6, 7]
                def v_ctr_rot():
                    i = VROT[qk_ctr[0] % 5]
                    qk_ctr[0] += 1
                    return psum(i)

                ab_ctr = [0]

                def emit_fused(c, v_t):
                    if _PROBE == "qkv":
                        return
                    for b in range(NBLK):
                        rs = act_m[b]
                        # row-packed broadcast matmuls, two at a time
                        ab_ps = {}
                        for ri in range(0, len(rs), 2):
                            for j, r in enumerate(rs[ri:ri + 2]):
                                ab = psum(3 + (ab_ctr[0] % 3))
                                ab_ctr[0] += 1
                                nc.tensor.matmul(
                                    ab[:], selB[32 * r:32 * r + 16, c, :],
                                    a16sb[b][32 * r:32 * r + 16, :],
                                    start=True, stop=True,
                                    skip_group_check=True,
                                    tile_position=(32 * r, 0))
                                ab_ps[r] = ab
                        r0 = rs[0]
                        accv = prpool.tile([128, BLK], F32, bufs=1,
                                           tag="f0", name="accv")
                        nc.vector.tensor_mul(accv[:], ab_ps[r0][:],
                                             v_t[(r0, b)][:])
                        if len(rs) == 1:
                            nc.vector.tensor_copy(fz_this[b][:, c, :], accv[:])
                        for j, r in enumerate(rs[1:]):
                            tmp = prpool.tile([128, BLK], F32, bufs=1,
                                              tag="f1", name="tmp")
                            nc.vector.tensor_mul(tmp[:], ab_ps[r][:],
                                                 v_t[(r, b)][:])
                            last = (j == len(rs) - 2)
                            nc.vector.tensor_add(
                                fz_this[b][:, c, :] if last else accv[:],
                                accv[:], tmp[:])

                # warm up two chunks of v so the PE has work during the
                # serial softmax chain
                vbuf = {0: emit_v(0), 1: emit_v(1)}

                # ---------- softmax (per block; DVE reads scores PSUM) ----
                attn = {}
                for b in (() if _PROBE == "qkv" else range(NBLK)):
                    act_w = [w for w in range(WIN) if n_sc[(w, b)] > 0]
                    sl = slice(64 * b, 64 * b + 64)
                    # DVE may read only one PSUM operand per instruction:
                    # copy the first window out, then chain maxes
                    mx = smpool.tile([64, BLK], F32, tag=f"mx{b}")
                    nc.vector.tensor_copy(mx[:], sc_ps[act_w[0]][sl, :])
                    for w in act_w[1:]:
                        nc.vector.tensor_tensor(
                            mx[:], mx[:], sc_ps[w][sl, :], op=ALU.max)
                    if len(act_w) < WIN:
                        # empty windows score 0: it participates in the max
                        nc.vector.tensor_scalar_max(mx[:], mx[:], 0.0)
                    for w in range(WIN):
                        a = smpool.tile([64, BLK], BF16, tag=f"at{w}{b}",
                                        name="attn")
                        if w in act_w:
                            ssub = smpool.tile([64, BLK], F32, bufs=2,
                                               tag="ss", name="ssub")
                            nc.vector.tensor_tensor(
                                ssub[:], sc_ps[w][sl, :], mx[:],
                                op=ALU.subtract)
                            nc.scalar.activation(a[:], ssub[:], AF.Exp,
                                                 scale=scale)
                        else:
                            nc.scalar.activation(a[:], mx[:], AF.Exp,
                                                 scale=-scale)
                        attn[(w, b)] = a
                    den = smpool.tile([64, BLK], F32, tag=f"mx{b}", name="den")
                    nc.vector.tensor_add(den[:], attn[(0, b)][:],
                                         attn[(1, b)][:])
                    nc.vector.tensor_add(den[:], den[:], attn[(2, b)][:])
                    rec = smpool.tile([64, BLK], F32R, tag=f"rec{b}")
                    with nc.allow_low_precision(reason="fp32r attn weights"):
                        nc.vector.reciprocal(rec[:], den[:])
                    for w in range(WIN):
                        nc.vector.tensor_mul(attn[(w, b)][:], attn[(w, b)][:],
                                             rec[:])

                # ---------- A16: per-source summed attn, packed in one bank
                # at partition offsets 32r (col tile_position) ----------
                a16sb = {}
                for b in (() if _PROBE == "qkv" else range(NBLK)):
                    a16 = psum(5)
                    for r in range(M):
                        if nact[b][r] == 0:
                            continue
                        srcs = SRC[r]
                        for i, (m, w) in enumerate(srcs):
                            nc.tensor.matmul(
                                a16[32 * r:32 * r + 16, :],
                                selA[:, PAIR_IDX[(m, w)], :],
                                attn[(w, b)][:],
                                start=(i == 0), stop=(i == len(srcs) - 1),
                                skip_group_check=True,
                                tile_position=(0, 32 * r))
                    t = smpool.tile([128, BLK], BF16, tag=f"a16_{b}",
                                    name="a16sb")
                    nc.scalar.activation(t[:], a16[:], AF.Identity)
                    a16sb[b] = t

                for c in range(NCH):
                    emit_fused(c, vbuf.pop(c))
                    if c + 2 < NCH:
                        vbuf[c + 2] = emit_v(c + 2)

            import contextlib
            rep_cm = (tc.For_i(0, trips, 1,
                               hint_engines=(mybir.EngineType.PE,
                                             mybir.EngineType.Activation,
                                             mybir.EngineType.DVE,
                                             mybir.EngineType.SP,
                                             mybir.EngineType.Pool),
                               staggered_reset=True)
                      if trips > 1 else contextlib.nullcontext())

            with rep_cm:
                # two fused-output buffer sets, alternating per unroll; the
                # D projection of set s is interleaved into the NEXT
                # iteration's pass 1 (cross-trip for the last unroll)
                fzsets = [[fzpool.tile([128, NCH, BLK], BF16, tag=f"fz{s}{b}",
                                       name=f"fz{s}{b}") for b in range(NBLK)]
                          for s in range(min(unroll, 2))]
                for u in range(unroll):
                    fz_prev = (fzsets[(u + 1) % 2] if repeat > 1 else None)
                    emit_iter(fzsets[u % 2] if repeat > 1 else fzsets[0],
                              fz_prev)

            # ---------- D: final output projection ----------
            # In the repeat loop each iteration's D is handled by the NEXT
            # iteration's pass-1 slots; only the last one remains here.
            # (For repeat=1 this is the only projection.)
            for dc in range(NCH):
                emit_D(dc, fzsets[(unroll - 1) % 2 if repeat > 1 else 0])

    nc.compile()
    _BUILD_CACHE[key] = nc
    return nc


def make_selw():
    sw = np.zeros((128, 127), np.float32)
    for p in range(128):
        sw[p, 62 + p // 64] = 1.0
    return sw


def make_selA():
    sa = np.zeros((64, M * WIN, 16), np.float32)
    for m in range(M):
        for w in range(WIN):
            for h in range(16):
                sa[16 * m + h, m * WIN + w, h] = 1.0
    return sa


def make_selB():
    # [128, NCH, 128]: row 32r+h holds, for chunk c, 0.25 * [h == 2c + j//64]
    sb = np.zeros((128, NCH, 128), np.float32)
    for r in range(M):
        for c in range(NCH):
            for j in range(128):
                sb[32 * r + 2 * c + j // 64, c, j] = 0.25
    return sb


def _vec_tile(v):
    return np.ascontiguousarray(np.asarray(v, np.float32).reshape(NCH, 128).T)


def _wf_tiles(Wf):
    """[D(out), dim(in)] fused weight -> [c, p(din%128), dk, j(dout%128)] bf16."""
    wt = np.asarray(Wf, np.float32).T                  # [din, dout]
    nk = wt.shape[0] // 128
    wt = wt.reshape(nk, 128, NCH, 128)                 # [dk, p, c, j]
    return wt.transpose(2, 1, 0, 3)                    # [c, p, dk, j]


def prepare_in_maps(inputs):
    names = [mm[0] for mm in MOD]
    emb = np.asarray(inputs["mod_emb"], np.float32)
    Wp = {pn: np.asarray(inputs[f"W{pn}"], np.float32) for pn in "qkvo"}
    bp = {pn: np.asarray(inputs[f"b{pn}"], np.float32) for pn in "qkvo"}

    shared = {}
    betaqk = np.zeros((128, M, 2, NCH), np.float32)
    betav = np.zeros((128, M, NCH), np.float32)
    tq, tk, tv = {}, {}, {}
    for i, nm in enumerate(names):
        Wm = np.asarray(inputs[f"W_{nm}"], np.float32)     # [D, dim]
        bm = np.asarray(inputs[f"b_{nm}"], np.float32) + emb[i]
        tq[i] = _wf_tiles(Wp["q"] @ Wm)
        tk[i] = _wf_tiles(Wp["k"] @ Wm)
        tv[i] = _wf_tiles(Wp["v"] @ Wm)
        betaqk[:, i, 0, :] = _vec_tile(Wp["q"] @ bm + bp["q"])
        betaqk[:, i, 1, :] = _vec_tile(Wp["k"] @ bm + bp["k"])
        betav[:, i, :] = _vec_tile(Wp["v"] @ bm + bp["v"])
    shared["betaqk"] = betaqk
    shared["betav"] = betav

    # pack all modalities along dk: [NCH, 128, 2, SNK, 128] / [NCH, 128, SNK, 128]
    wqk_all = np.concatenate(
        [np.stack([tq[m], tk[m]], axis=2) for m in range(M)], axis=3)
    shared["Wqk"] = np.ascontiguousarray(wqk_all).astype(ml_dtypes.bfloat16)
    wv_all = np.concatenate([tv[m] for m in range(M)], axis=2)
    shared["Wvf"] = np.ascontiguousarray(wv_all).astype(ml_dtypes.bfloat16)

    wo = Wp["o"].T.reshape(NCH, 128, NCH, 128)             # [dk, p, c, j]
    shared["WoT"] = np.ascontiguousarray(
        wo.transpose(2, 1, 0, 3)).astype(ml_dtypes.bfloat16)
    shared["bo"] = _vec_tile(bp["o"])
    shared["selw"] = make_selw().astype(ml_dtypes.bfloat16)
    shared["selA"] = make_selA().astype(ml_dtypes.bfloat16)
    shared["selB"] = make_selB().astype(ml_dtypes.bfloat16)

    in_maps = []
    for core in range(8):
        b, par = core // 2, core % 2
        im = dict(shared)
        for i, nm in enumerate(names):
            x = np.asarray(inputs[nm], np.float32)[b, par::2][:NLOC[i]]
            xt = x.T.reshape(NK[i], 128, NLOC[i]).transpose(1, 0, 2)
            im[f"xT{i}"] = np.ascontiguousarray(xt).astype(ml_dtypes.bfloat16)
        in_maps.append(im)
    return in_maps


def kernel(**inputs):
    inputs = {k: np.asarray(v) for k, v in inputs.items()}
    scale = float(1.0 / (np.sqrt(HD) * abs(float(inputs["temperature"]))))
    nc = build(scale, repeat=1)
    in_maps = prepare_in_maps(inputs)
    res = run_bass_kernel_spmd(nc, in_maps, list(range(8)))
    out = np.zeros((B, S, D), np.float32)
    for core in range(8):
        b, par = core // 2, core % 2
        out[b, par::2, :] = np.asarray(res.results[core]["yT"],
                                       np.float32).T
    return out
